# revision 1
# baseline (speedup 1.0000x reference)
"""GPT-J attention (B=2, S=2048, D=4096, 16 heads x 256, partial RoPE 64) on 8 trn2 cores.

Sharding: tensor-parallel over heads — each core owns 2 heads (Wq/Wk/Wv column
slices, Wo row slice), computes its partial out-projection, on-device
ReduceScatter sums partials and leaves each core with a 512-row shard of the
[B*S, D] output; host concatenates shards.

Device kernel layout strategy (per core):
  - hidden_states transposed on host to hsT [B, D, S] so the model dim (the
    matmul contraction) lies on SBUF partitions.
  - QKV projection: out = W_chunk.T @ hsT_chunk accumulated over d-chunks.
    Weights are streamed in NG d-groups; partial sums accumulated in SBUF.
  - QT/KT produced feature-major [hd, s]; V produced natural [s, hd] (by
    swapping stationary/moving operands) so PV can use V as stationary.
  - RoPE: rot = C * q + S2 * swap(q); swap(q) via a tiny PE matmul with a
    pair-swap permutation matrix; C/S2 precomputed on host, laid out [64, S].
  - Attention per (b, head): flash-style over 512-query macro tiles, scores
    via PE, exp on ACT (fused 1/16 scale; causal mask added to diagonal
    tiles from host-built additive mask patterns), unnormalized P transposed
    via PE (identity) to feed PV; softmax normalization deferred to the
    out-projection evacuation (tensor_scalar by 1/rowsum, rowsum collected
    free via activation accum_out).
  - Out-projection: y_partial = attnT.T @ WoT accumulated over local hd,
    normalized at PSUM evacuation, ReduceScatter(add) across 8 cores.

All matmuls run in float32r (fp32 bits, replicated PE mode: full rate at
free-dim >= 256) unless MM_DTYPE says otherwise.
"""

import os
import sys

import numpy as np

sys.path.insert(0, "/opt/trn_rl_repo")

# ---------------------------------------------------------------- constants
B = 2
S = 2048
D = 4096
NH = 16
HD = 256
ROT = 64
MAX_POS = 2048
N_CORES = 8
HPC = NH // N_CORES          # heads per core = 2
HDL = HPC * HD               # local head width = 512

SC = 512                     # s-chunk (projection, q-macro, k-tile width)
QS = 128                     # q-subtile
NEG = -1.0e30


def _cfg_full():
    return dict(B=B, S=S, D=D, HPC=HPC, HD=HD, ROT=ROT)


# ---------------------------------------------------------------- bass build

def build_nc(cfg, use_collective=True, n_cores=N_CORES, mm_dtype="float32r", debug_taps=False):
    import concourse.tile as tile
    from concourse import bacc, mybir

    fp32 = mybir.dt.float32
    mdt = getattr(mybir.dt, mm_dtype)

    Bc, Sc, Dc, HPCc, HDc, ROTc = (
        cfg["B"], cfg["S"], cfg["D"], cfg["HPC"], cfg["HD"], cfg["ROT"])
    HDLc = HPCc * HDc                    # local head width
    NHC = HDLc // 128                    # local hd chunks (4)
    NSC = Sc // SC                       # s-chunks (4)
    NDC = Dc // 128                      # d chunks (32)
    DG = 1024 if Dc % 1024 == 0 else Dc  # d-group size
    NG = Dc // DG                        # d-groups
    GDC = DG // 128                      # d-chunks per group (8)
    SHARD = (Bc * Sc) // n_cores if use_collective else Bc * Sc

    nc = bacc.Bacc(num_devices=n_cores)

    # inputs (per-core)
    hsT_e = nc.declare_dram_parameter("hsT", [Bc, Dc, Sc], mdt, isOutput=False)
    wqT_e = nc.declare_dram_parameter("wqT", [Dc, HDLc], mdt, isOutput=False)
    wkT_e = nc.declare_dram_parameter("wkT", [Dc, HDLc], mdt, isOutput=False)
    wvT_e = nc.declare_dram_parameter("wvT", [Dc, HDLc], mdt, isOutput=False)
    woT_e = nc.declare_dram_parameter("woT", [HDLc, Dc], mdt, isOutput=False)
    cos_e = nc.declare_dram_parameter("cosb", [Bc, ROTc, Sc], mdt, isOutput=False)
    sin_e = nc.declare_dram_parameter("sinb", [Bc, ROTc, Sc], mdt, isOutput=False)
    msk_e = nc.declare_dram_parameter("masks", [QS, 4, SC], fp32, isOutput=False)
    psw_e = nc.declare_dram_parameter("pswap", [128, ROTc], mdt, isOutput=False)
    idn_e = nc.declare_dram_parameter("ident", [128, 128], fp32, isOutput=False)

    y_e = nc.declare_dram_parameter("y", [SHARD, Dc], fp32, isOutput=True)
    if debug_taps:
        dbg = {}
        for nm, shp in [("dbg_qt", [NHC, 128, Sc]), ("dbg_kt", [NHC, 128, Sc]),
                        ("dbg_v", [Sc, HDLc]), ("dbg_atn", [NHC, 128, Sc]),
                        ("dbg_recip", [128, HPCc, Sc // QS])]:
            dbg[nm] = nc.declare_dram_parameter(nm, shp, fp32, isOutput=True)
    if use_collective:
        y_part = nc.dram_tensor("y_part", [Bc * Sc, Dc], fp32)
        rs_out = nc.dram_tensor("rs_out", [SHARD, Dc], fp32)

    def mm(ps, lhsT, rhs, start, stop):
        nc.tensor.matmul(ps, lhsT, rhs, start=start, stop=stop)

    with tile.TileContext(nc) as tc:
        with tc.tile_pool(name="const", bufs=1) as constp:
            masks = constp.tile([QS, 4, SC], fp32)
            nc.sync.dma_start(masks[:], msk_e[:])
            pswap = constp.tile([128, ROTc], mdt)
            nc.sync.dma_start(pswap[:], psw_e[:])
            ident = constp.tile([128, 128], fp32)
            nc.sync.dma_start(ident[:], idn_e[:])

            with (
                tc.tile_pool(name="qkv", bufs=1) as qkvp,
                tc.tile_pool(name="atn", bufs=1) as atnp,
                tc.tile_pool(name="rsum", bufs=1) as rsump,
            ):
                for b in range(Bc):
                    # persistent per-b tensors (slots reused across b)
                    QT = [qkvp.tile([128, Sc], mdt, tag=f"QT{c}", name=f"QT{c}") for c in range(NHC)]
                    KT = [qkvp.tile([128, Sc], mdt, tag=f"KT{c}", name=f"KT{c}") for c in range(NHC)]
                    V = [qkvp.tile([128, HDLc], mdt, tag=f"V{ss}", name=f"V{ss}")
                         for ss in range(Sc // 128)]
                    recip = rsump.tile([128, HPCc, Sc // QS], fp32, tag="recip")

                    # ---------------- phase A: QKV projection ----------------
                    with (
                        tc.tile_pool(name="wts", bufs=1) as wp,
                        tc.tile_pool(name="hst", bufs=2) as hp,
                        tc.tile_pool(name="pjps", bufs=1, space="PSUM") as pjps,
                    ):
                        for g in range(NG):
                            gsl = slice(g * DG, (g + 1) * DG)
                            wq = wp.tile([128, GDC, HDLc], mdt, tag="wq")
                            wk = wp.tile([128, GDC, HDLc], mdt, tag="wk")
                            wv = wp.tile([128, GDC, HDLc], mdt, tag="wv")
                            nc.sync.dma_start(
                                wq[:], wqT_e[gsl, :].rearrange("(j p) f -> p j f", p=128))
                            nc.sync.dma_start(
                                wk[:], wkT_e[gsl, :].rearrange("(j p) f -> p j f", p=128))
                            nc.sync.dma_start(
                                wv[:], wvT_e[gsl, :].rearrange("(j p) f -> p j f", p=128))
                            for sc in range(NSC):
                                ssl = slice(sc * SC, (sc + 1) * SC)
                                GH = GDC // 2
                                hst_a = hp.tile([128, GH, SC], mdt, tag="hst")
                                hst_b = hp.tile([128, GH, SC], mdt, tag="hst")
                                g0 = slice(g * DG, g * DG + GH * 128)
                                g1 = slice(g * DG + GH * 128, (g + 1) * DG)
                                nc.sync.dma_start(
                                    hst_a[:],
                                    hsT_e[b, g0, ssl].rearrange("(j p) f -> p j f", p=128))
                                nc.sync.dma_start(
                                    hst_b[:],
                                    hsT_e[b, g1, ssl].rearrange("(j p) f -> p j f", p=128))

                                def hst(dc, _a=hst_a, _b=hst_b, _gh=GH):
                                    return _a if dc < _gh else _b

                                def hsti(dc, _gh=GH):
                                    return dc % _gh
                                # Q/K: accumulate over this group's d-chunks
                                psq = [pjps.tile([128, SC], fp32, tag=f"psq{h}", name=f"psq{h}")
                                       for h in range(NHC)]
                                psk = [pjps.tile([128, SC], fp32, tag=f"psk{h}", name=f"psk{h}")
                                       for h in range(NHC)]
                                for dc in range(GDC):
                                    for h in range(NHC):
                                        hsl = slice(h * 128, (h + 1) * 128)
                                        mm(psq[h][:], wq[:, dc, hsl], hst(dc)[:, hsti(dc), :],
                                           start=(dc == 0), stop=(dc == GDC - 1))
                                        mm(psk[h][:], wk[:, dc, hsl], hst(dc)[:, hsti(dc), :],
                                           start=(dc == 0), stop=(dc == GDC - 1))
                                for h in range(NHC):
                                    if g == 0:
                                        nc.vector.tensor_copy(QT[h][:, ssl], psq[h][:])
                                        nc.vector.tensor_copy(KT[h][:, ssl], psk[h][:])
                                    else:
                                        nc.vector.tensor_add(
                                            QT[h][:, ssl], QT[h][:, ssl], psq[h][:])
                                        nc.vector.tensor_add(
                                            KT[h][:, ssl], KT[h][:, ssl], psk[h][:])
                                # V: stationary = hsT s-subtiles
                                psv = [pjps.tile([128, HDLc], fp32, tag=f"psq{ss}", name=f"psv{ss}")
                                       for ss in range(SC // 128)]
                                for dc in range(GDC):
                                    for ss in range(SC // 128):
                                        ssub = slice(ss * 128, (ss + 1) * 128)
                                        mm(psv[ss][:], hst(dc)[:, hsti(dc), ssub], wv[:, dc, :],
                                           start=(dc == 0), stop=(dc == GDC - 1))
                                for ss in range(SC // 128):
                                    vi = sc * (SC // 128) + ss
                                    if g == 0:
                                        nc.vector.tensor_copy(V[vi][:], psv[ss][:])
                                    else:
                                        nc.vector.tensor_add(V[vi][:], V[vi][:], psv[ss][:])

                    # ---------------- RoPE on QT/KT rot rows ----------------
                    with (
                        tc.tile_pool(name="trig", bufs=1) as trigp,
                        tc.tile_pool(name="rope", bufs=4) as ropep,
                        tc.tile_pool(name="rops", bufs=2, space="PSUM") as ropsp,
                    ):
                        cosb = trigp.tile([ROTc, Sc], mdt, tag="cos")
                        sinb = trigp.tile([ROTc, Sc], mdt, tag="sin")
                        nc.sync.dma_start(cosb[:], cos_e[b])
                        nc.sync.dma_start(sinb[:], sin_e[b])
                        for t in (QT, KT):
                            for hch in range(0, NHC, HDc // 128):
                                for sc in range(NSC):
                                    ssl = slice(sc * SC, (sc + 1) * SC)
                                    sw = ropsp.tile([ROTc, SC], fp32, tag="sw")
                                    mm(sw[:], pswap[:, :], t[hch][:, ssl],
                                       start=True, stop=True)
                                    t1 = ropep.tile([ROTc, SC], mdt, tag="t1")
                                    t2 = ropep.tile([ROTc, SC], mdt, tag="t2")
                                    nc.vector.tensor_tensor(
                                        t1[:], sw[:], sinb[:, ssl],
                                        op=mybir.AluOpType.mult)
                                    nc.vector.tensor_tensor(
                                        t2[:], t[hch][0:ROTc, ssl], cosb[:, ssl],
                                        op=mybir.AluOpType.mult)
                                    nc.vector.tensor_add(t[hch][0:ROTc, ssl],
                                                         t1[:], t2[:])

                    # ---------------- phase B: attention ----------------
                    ATN = [atnp.tile([128, Sc], mdt, tag=f"ATN{c}", name=f"ATN{c}") for c in range(NHC)]
                    with (
                        tc.tile_pool(name="pbuf", bufs=1) as pbufp,
                        tc.tile_pool(name="ptsb", bufs=3) as ptsbp,
                        tc.tile_pool(name="scps", bufs=2, space="PSUM") as scps,
                        tc.tile_pool(name="ptps", bufs=2, space="PSUM") as ptps,
                        tc.tile_pool(name="atps", bufs=2, space="PSUM") as atps,
                        tc.tile_pool(name="rs", bufs=8) as rsp,
                    ):
                        for h in range(HPC):
                            c0 = h * (HDc // 128)      # first hd chunk of head
                            for qm in range(NSC):
                                nkt = qm + 1           # valid k-tiles of 512
                                nkc = nkt * (SC // 128)  # valid k-chunks of 128
                                P = [pbufp.tile([128, Sc], fp32, tag=f"P{qs}", name=f"P{qs}")
                                     for qs in range(SC // QS)]
                                for qs in range(SC // QS):
                                    g = qm * (SC // QS) + qs
                                    qsl = slice(g * QS, (g + 1) * QS)
                                    racc = rsp.tile([128, 1], fp32, tag=f"racc{qs}")
                                    for kt in range(nkt):
                                        ksl = slice(kt * SC, (kt + 1) * SC)
                                        ss = scps.tile([128, SC], fp32, tag="ss")
                                        mm(ss[:], QT[c0][:, qsl], KT[c0][:, ksl],
                                           start=True, stop=False)
                                        mm(ss[:], QT[c0 + 1][:, qsl], KT[c0 + 1][:, ksl],
                                           start=False, stop=True)
                                        if kt == nkt - 1:
                                            nc.vector.tensor_add(
                                                ss[:], ss[:], masks[:, qs, :])
                                        if kt == 0:
                                            acc_ap = racc[:]
                                        else:
                                            rpart = rsp.tile([128, 1], fp32, tag="rpart")
                                            acc_ap = rpart[:]
                                        nc.scalar.activation(
                                            P[qs][:, ksl], ss[:],
                                            mybir.ActivationFunctionType.Exp,
                                            bias=0.0, scale=1.0 / 16.0,
                                            accum_out=acc_ap)
                                        if kt > 0:
                                            nc.vector.tensor_add(
                                                racc[:], racc[:], rpart[:])
                                    nc.vector.reciprocal(
                                        recip[:, h, g:g + 1], racc[:])
                                # transpose P + PV
                                atn_ps = [atps.tile([128, SC], fp32, tag=f"atn{hh}", name=f"atnps{hh}")
                                          for hh in range(HDc // 128)]
                                for kc in range(nkc):
                                    kcl = slice(kc * 128, (kc + 1) * 128)
                                    ptp = ptps.tile([128, SC], fp32, tag="ptp")
                                    for qs in range(SC // QS):
                                        nc.tensor.transpose(
                                            ptp[:, qs * 128:(qs + 1) * 128],
                                            P[qs][:, kcl], ident[:])
                                    pts = ptsbp.tile([128, SC], mdt, tag="pts")
                                    nc.vector.tensor_copy(pts[:], ptp[:])
                                    for hh in range(HDc // 128):
                                        mm(atn_ps[hh][:],
                                           V[kc][:, h * HDc + hh * 128:
                                                 h * HDc + (hh + 1) * 128],
                                           pts[:],
                                           start=(kc == 0), stop=(kc == nkc - 1))
                                for hh in range(HDc // 128):
                                    nc.vector.tensor_copy(
                                        ATN[c0 + hh][:, qm * SC:(qm + 1) * SC],
                                        atn_ps[hh][:])

                    if debug_taps and b == 0:
                        for c in range(NHC):
                            nc.sync.dma_start(dbg["dbg_qt"][c], QT[c][:])
                            nc.sync.dma_start(dbg["dbg_kt"][c], KT[c][:])
                            nc.sync.dma_start(dbg["dbg_atn"][c], ATN[c][:])
                        for ss in range(Sc // 128):
                            nc.sync.dma_start(
                                dbg["dbg_v"][ss * 128:(ss + 1) * 128, :], V[ss][:])
                        nc.sync.dma_start(dbg["dbg_recip"][:], recip[:])

                    # ---------------- phase C: out projection ----------------
                    with (
                        tc.tile_pool(name="wo", bufs=2) as wop,
                        tc.tile_pool(name="ysb", bufs=4) as ysbp,
                        tc.tile_pool(name="yps", bufs=3, space="PSUM") as ypsp,
                    ):
                        NCH = HDc // 128   # chunks per head
                        dst = y_part if use_collective else y_e
                        for oc in range(Dc // SC):
                            ocl = slice(oc * SC, (oc + 1) * SC)
                            woc = wop.tile([128, NHC, SC], mdt, tag="woc")
                            nc.sync.dma_start(
                                woc[:],
                                woT_e[:, ocl].rearrange("(c p) f -> p c f", p=128))
                            for sg in range(Sc // QS):
                                ssl = slice(sg * 128, (sg + 1) * 128)
                                ysb = ysbp.tile([128, SC], fp32, tag="ysb")
                                for h in range(HPCc):
                                    yps = ypsp.tile([128, SC], fp32,
                                                    tag=f"yps{h}", name=f"yps{h}")
                                    for cc in range(NCH):
                                        c = h * NCH + cc
                                        mm(yps[:], ATN[c][:, ssl], woc[:, c, :],
                                           start=(cc == 0), stop=(cc == NCH - 1))
                                    if h == 0:
                                        nc.vector.tensor_scalar(
                                            out=ysb[:], in0=yps[:],
                                            scalar1=recip[:, 0, sg:sg + 1],
                                            scalar2=None,
                                            op0=mybir.AluOpType.mult)
                                    else:
                                        nc.vector.scalar_tensor_tensor(
                                            out=ysb[:], in0=yps[:],
                                            scalar=recip[:, h, sg:sg + 1],
                                            in1=ysb[:],
                                            op0=mybir.AluOpType.mult,
                                            op1=mybir.AluOpType.add)
                                nc.sync.dma_start(
                                    dst[b * Sc + sg * 128:b * Sc + (sg + 1) * 128,
                                        ocl],
                                    ysb[:])

            if use_collective:
                nc.gpsimd.collective_compute(
                    "ReduceScatter",
                    mybir.AluOpType.add,
                    replica_groups=[list(range(n_cores))],
                    ins=[y_part[:]],
                    outs=[rs_out[:]],
                )
                nc.sync.dma_start(y_e[:], rs_out[:])

    nc.compile()
    return nc


# ---------------------------------------------------------------- host prep

def _sinusoidal_np(num_pos, dim):
    inv_freq = 1.0 / (10000.0 ** (np.arange(0, dim, 2, dtype=np.float32) / dim))
    t = np.arange(num_pos, dtype=np.float32)[:, None] * inv_freq[None, :]
    return np.cos(t).astype(np.float32), np.sin(t).astype(np.float32)  # [P, dim//2]


def _host_arrays(hs, Wq, Wk, Wv, Wo, position_ids, cfg, n_cores):
    """Build the shared + per-core input arrays."""
    Bc, Sc, Dc, HPCc, HDc, ROTc = (
        cfg["B"], cfg["S"], cfg["D"], cfg["HPC"], cfg["HD"], cfg["ROT"])
    HDLc = HPCc * HDc
    hsT = np.ascontiguousarray(hs.transpose(0, 2, 1)).astype(np.float32)

    cos_t, sin_t = _sinusoidal_np(max(MAX_POS, Sc), ROTc)   # [P, ROT//2]
    pos = np.asarray(position_ids).astype(np.int64)         # [B, S]
    cosg = cos_t[pos]                                       # [B, S, 32]
    sing = sin_t[pos]
    cosb = np.repeat(cosg.transpose(0, 2, 1), 2, axis=1)    # [B, 64, S]
    sinb_r = np.repeat(sing.transpose(0, 2, 1), 2, axis=1)
    sgn = np.ones((ROTc, 1), np.float32)
    sgn[0::2] = -1.0
    sinb = (sinb_r * sgn).astype(np.float32)
    cosb = np.ascontiguousarray(cosb).astype(np.float32)

    masks = np.zeros((4, QS, SC), np.float32)
    qq = np.arange(QS)[:, None]
    kk = np.arange(SC)[None, :]
    for m in range(4):
        masks[m] = np.where(kk <= m * QS + qq, 0.0, NEG)
    masks = np.ascontiguousarray(masks.transpose(1, 0, 2))  # [QS, 4, SC]

    pswap = np.zeros((128, ROTc), np.float32)
    for f in range(ROTc // 2):
        pswap[2 * f + 1, 2 * f] = 1.0
        pswap[2 * f, 2 * f + 1] = 1.0
    ident = np.eye(128, dtype=np.float32)

    shared = dict(hsT=hsT, cosb=cosb, sinb=sinb, masks=masks,
                  pswap=pswap, ident=ident)
    per_core = []
    for c in range(n_cores):
        csl = slice(c * HDLc, (c + 1) * HDLc)
        per_core.append(dict(
            wqT=np.ascontiguousarray(Wq[csl, :].T),
            wkT=np.ascontiguousarray(Wk[csl, :].T),
            wvT=np.ascontiguousarray(Wv[csl, :].T),
            woT=np.ascontiguousarray(Wo[:, csl].T),
            **shared,
        ))
    return per_core


def _numpy_reference(hidden_states, Wq, Wk, Wv, Wo, layer_past_k, layer_past_v,
                     attention_mask, position_ids, new_key_loc, new_value_loc,
                     valid_key_indices, valid_value_indices, bucket_size):
    """Slow but general fallback (mirrors reference.py in numpy fp32)."""
    hs = np.asarray(hidden_states, np.float32)
    Bc, Sc, Dc = hs.shape
    q = (hs @ np.asarray(Wq).T).reshape(Bc, Sc, NH, HD)
    k = (hs @ np.asarray(Wk).T).reshape(Bc, Sc, NH, HD)
    v = (hs @ np.asarray(Wv).T).reshape(Bc, Sc, NH, HD)

    cos_t, sin_t = _sinusoidal_np(MAX_POS, ROT)
    pos = np.asarray(position_ids).astype(np.int64)
    c_ = cos_t[pos][:, :, None, :]      # [B,S,1,32]
    s_ = sin_t[pos][:, :, None, :]

    def rot(x):
        xr = x[..., :ROT].reshape(Bc, Sc, NH, ROT // 2, 2)
        x0, x1 = xr[..., 0], xr[..., 1]
        o0 = c_ * x0 - s_ * x1
        o1 = s_ * x0 + c_ * x1
        out = np.stack([o0, o1], axis=-1).reshape(Bc, Sc, NH, ROT)
        return np.concatenate([out, x[..., ROT:]], axis=-1)

    q, k = rot(q), rot(k)
    nk = np.asarray(layer_past_k, np.float32).copy()
    nv = np.asarray(layer_past_v, np.float32).copy()
    nk[np.asarray(new_key_loc)] = k.reshape(Bc * Sc, 1, NH, HD)
    nv[np.asarray(new_value_loc)] = v.reshape(Bc * Sc, 1, NH, HD)
    kg = nk[np.asarray(valid_key_indices)].reshape(
        Bc, bucket_size, NH, HD).transpose(0, 2, 1, 3)
    vg = nv[np.asarray(valid_value_indices)].reshape(
        Bc, bucket_size, NH, HD).transpose(0, 2, 1, 3)
    qh = q.transpose(0, 2, 1, 3)
    scores = np.einsum("bhqd,bhkd->bhqk", qh, kg)
    causal = np.tril(np.ones((MAX_POS, MAX_POS), bool))[
        bucket_size - Sc:bucket_size, :bucket_size]
    scores = np.where(causal, scores, np.float32(np.finfo(np.float32).min))
    scores = scores / np.float32(np.sqrt(HD)) + np.asarray(attention_mask, np.float32)
    scores = scores - scores.max(-1, keepdims=True)
    p = np.exp(scores)
    p = p / p.sum(-1, keepdims=True)
    attn = np.einsum("bhqk,bhkd->bhqd", p, vg)
    attn = attn.transpose(0, 2, 1, 3).reshape(Bc, Sc, Dc)
    return (attn @ np.asarray(Wo).T).astype(np.float32)


def _fast_path_ok(layer_past_k, layer_past_v, attention_mask, new_key_loc,
                  new_value_loc, valid_key_indices, valid_value_indices,
                  bucket_size, hs_shape):
    Bc, Sc, Dc = hs_shape
    if (Bc, Sc, Dc) != (B, S, D) or int(bucket_size) != S:
        return False
    ar = np.arange(Bc * Sc)
    for idx in (new_key_loc, new_value_loc, valid_key_indices, valid_value_indices):
        a = np.asarray(idx)
        if a.shape != (Bc * Sc,) or not np.array_equal(a, ar):
            return False
    if np.any(np.asarray(attention_mask) != 0):
        return False
    return True


_NC_CACHE = {}


def _get_nc(use_collective=True):
    key = ("full", use_collective)
    if key not in _NC_CACHE:
        _NC_CACHE[key] = build_nc(_cfg_full(), use_collective=use_collective,
                                  n_cores=N_CORES)
    return _NC_CACHE[key]


def kernel(**inputs):
    hs = np.asarray(inputs["hidden_states"], np.float32)
    fast = _fast_path_ok(
        inputs["layer_past_k"], inputs["layer_past_v"], inputs["attention_mask"],
        inputs["new_key_loc"], inputs["new_value_loc"],
        inputs["valid_key_indices"], inputs["valid_value_indices"],
        inputs["bucket_size"], hs.shape)
    if not fast:
        return _numpy_reference(**inputs)

    from concourse.bass_utils import run_bass_kernel_spmd

    use_collective = os.environ.get("KERNEL_NO_COLLECTIVE", "") != "1"
    nc = _get_nc(use_collective)
    in_maps = _host_arrays(
        hs, np.asarray(inputs["Wq"], np.float32),
        np.asarray(inputs["Wk"], np.float32),
        np.asarray(inputs["Wv"], np.float32),
        np.asarray(inputs["Wo"], np.float32),
        inputs["position_ids"], _cfg_full(), N_CORES)
    res = run_bass_kernel_spmd(nc, in_maps, list(range(N_CORES)))
    outs = [res.results[c]["y"] for c in range(N_CORES)]
    if use_collective:
        y = np.concatenate(outs, axis=0)
    else:
        y = np.sum(np.stack(outs), axis=0)
    return y.reshape(B, S, D).astype(np.float32)



# revision 7
# speedup vs baseline: 1.3151x; 1.3151x over previous
"""GPT-J attention (B=2, S=2048, D=4096, 16 heads x 256, partial RoPE 64) on 8 trn2 cores.

Tensor-parallel over heads: each core owns 2 heads (Wq/Wk/Wv column slices,
Wo row slice). Per-oc-column bf16 ReduceScatter sums the partial out-projections
and leaves each core a 512-token shard of [B*S, D]; host concatenates shards.

v2 kernel strategy (all matmul inputs bf16, PSUM accumulation fp32):
  - Phase A per b: QKV projections accumulate over ALL 32 d-chunks directly in
    PSUM (no SBUF partial-sum round trips). Five passes per 512-token tile
    (Q01, Q23, K01, K23, V) using <= 6 PSUM banks with evacuation (alternating
    ACT/DVE) pipelined behind the next pass. Weights and hsT streamed in
    host-pre-swizzled partition-contiguous blocks. RoPE folded in per tile.
  - Phase B: scores computed TRANSPOSED (ssT[k,q] = KT_chunk^T @ QT) so the
    exp output is directly P^T, feeding PV with V as stationary - no PE
    transposes, no P copies. Row sums via a ones-column matmul into PSUM
    (partitions 0/32/64/96 of 2 banks), then reciprocal + one small PE
    transpose gives recip in q-partition layout. Softmax normalization is
    deferred to the phase-C evacuation. Software-pipelined: scores(kc+1)
    emitted before PV(kc).
  - Phase C per b, oc-outer: y partials accumulate per (sg,h) in PSUM;
    h=0 evacuated by ACT (activation Copy with per-partition recip scale),
    h=1 by DVE fused mult-add. bf16 y_part written per oc block; after b=1
    finishes an oc block, a ReduceScatter(add) for just that [4096,512] block
    fires, overlapping the remaining compute.
"""

import os
import sys

import numpy as np

sys.path.insert(0, "/opt/trn_rl_repo")

# ---------------------------------------------------------------- constants
B = 2
S = 2048
D = 4096
NH = 16
HD = 256
ROT = 64
MAX_POS = 2048
N_CORES = 8
HPC = NH // N_CORES          # heads per core = 2
HDL = HPC * HD               # local head width = 512

SC = 512                     # token tile / q macro tile / moving width
NEG = -1.0e30


def _cfg_full():
    return dict(B=B, S=S, D=D, HPC=HPC, HD=HD, ROT=ROT)


# ---------------------------------------------------------------- bass build

def build_nc(cfg, use_collective=True, n_cores=N_CORES):
    import concourse.tile as tile
    from concourse import bacc, mybir

    fp32 = mybir.dt.float32
    bf16 = mybir.dt.bfloat16

    Bc, Sc, Dc, HPCc, HDc, ROTc = (
        cfg["B"], cfg["S"], cfg["D"], cfg["HPC"], cfg["HD"], cfg["ROT"])
    HDLc = HPCc * HDc                    # local head width (512)
    NHC = HDLc // 128                    # local hd chunks (4)
    NSC = Sc // SC                       # 512-token tiles per b (4)
    NDC = Dc // 128                      # d chunks (32)
    NG = NDC // 8                        # streamed weight groups (4)
    NOC = Dc // SC                       # out-proj column chunks (8)
    NKC = Sc // 128                      # k chunks per b (16)
    SHARD = (Bc * Sc) // n_cores if use_collective else Bc * Sc

    nc = bacc.Bacc(num_devices=n_cores)

    # inputs (per-core, host-pre-swizzled for contiguous DMA)
    hs_e = nc.declare_dram_parameter("hs_s", [Bc, NSC, 2, 128, NDC // 2, SC],
                                     bf16, isOutput=False)
    wq_e = nc.declare_dram_parameter("wq_s", [2, NG, 128, 8, HDLc // 2],
                                     bf16, isOutput=False)
    wk_e = nc.declare_dram_parameter("wk_s", [2, NG, 128, 8, HDLc // 2],
                                     bf16, isOutput=False)
    wv_e = nc.declare_dram_parameter("wv_s", [NG, 128, 8, HDLc],
                                     bf16, isOutput=False)
    wo_e = nc.declare_dram_parameter("wo_s", [NOC, 128, NHC, SC],
                                     bf16, isOutput=False)
    cos_e = nc.declare_dram_parameter("cosb", [Bc, ROTc, Sc], bf16, isOutput=False)
    sin_e = nc.declare_dram_parameter("sinb", [Bc, ROTc, Sc], bf16, isOutput=False)
    msk_e = nc.declare_dram_parameter("masksT", [128, 4, SC], bf16, isOutput=False)
    psw_e = nc.declare_dram_parameter("pswap", [128, ROTc], bf16, isOutput=False)
    id8_e = nc.declare_dram_parameter("ident8", [8, 8], fp32, isOutput=False)
    one_e = nc.declare_dram_parameter("onesc", [128, 1], bf16, isOutput=False)

    if use_collective:
        y_e = nc.declare_dram_parameter("y", [SHARD, Dc], bf16, isOutput=True)
        y_part = nc.dram_tensor("y_part", [NOC, Bc * Sc, SC], bf16)
        rs_out = nc.dram_tensor("rs_out", [NOC, SHARD, SC], bf16)
    else:
        y_e = nc.declare_dram_parameter("y", [NOC, Bc * Sc, SC], bf16,
                                        isOutput=True)

    def mm(ps, lhsT, rhs, start, stop):
        nc.tensor.matmul(ps, lhsT, rhs, start=start, stop=stop)

    with tile.TileContext(nc) as tc:
        with tc.tile_pool(name="const", bufs=1) as constp:
            masks = constp.tile([128, 4, SC], bf16)
            nc.sync.dma_start(masks[:], msk_e[:])
            pswap = constp.tile([128, ROTc], bf16)
            nc.sync.dma_start(pswap[:], psw_e[:])
            ident8 = constp.tile([8, 8], fp32)
            nc.sync.dma_start(ident8[:], id8_e[:])
            ones = constp.tile([128, 1], bf16)
            nc.sync.dma_start(ones[:], one_e[:])

            with (
                tc.tile_pool(name="qkv", bufs=1) as qkvp,      # QT/KT/V one b
                tc.tile_pool(name="atn", bufs=1) as atnp,      # ATN one b
                tc.tile_pool(name="rcp", bufs=1) as rcpp,      # recip per b
                tc.tile_pool(name="trig", bufs=1) as trigp,
            ):
                for b in range(Bc):
                    # ============ phase A: QKV projection (PSUM-resident) ====
                    QT = [qkvp.tile([128, Sc], bf16, tag=f"QT{c}", name=f"QT{c}") for c in range(NHC)]
                    KT = [qkvp.tile([128, Sc], bf16, tag=f"KT{c}", name=f"KT{c}") for c in range(NHC)]
                    V = [qkvp.tile([128, HDLc], bf16, tag=f"V{k}", name=f"V{k}") for k in range(NKC)]

                    cosb = trigp.tile([ROTc, Sc], bf16, tag="cos")
                    sinb = trigp.tile([ROTc, Sc], bf16, tag="sin")
                    nc.sync.dma_start(cosb[:], cos_e[b])
                    nc.sync.dma_start(sinb[:], sin_e[b])

                    with (
                        tc.tile_pool(name="hst", bufs=3) as hp,
                        tc.tile_pool(name="wqk", bufs=3) as wqkp,
                        tc.tile_pool(name="wvs", bufs=2) as wvp,
                        tc.tile_pool(name="pjps", bufs=1, space="PSUM") as pjps,
                        tc.tile_pool(name="rops", bufs=2, space="PSUM") as ropsp,
                        tc.tile_pool(name="ropb", bufs=2) as ropbp,
                    ):
                        HND = NDC // 2
                        for st in range(NSC):
                            ssl = slice(st * SC, (st + 1) * SC)
                            ha = hp.tile([128, HND, SC], bf16, tag="hst")
                            hb = hp.tile([128, HND, SC], bf16, tag="hst")
                            nc.sync.dma_start(ha[:], hs_e[b, st, 0])
                            nc.sync.dma_start(hb[:], hs_e[b, st, 1])

                            def hst(dc):
                                return (ha if dc < HND else hb)[:, dc % HND, :]

                            def rope(t, c):
                                # rotate rows 0:ROT of t[c] at columns ssl
                                sw = ropsp.tile([ROTc, SC], fp32, tag="rp")
                                mm(sw[:], pswap[:, :], t[c][:, ssl],
                                   start=True, stop=True)
                                t1 = ropbp.tile([ROTc, SC], bf16, tag="t1")
                                t2 = ropbp.tile([ROTc, SC], bf16, tag="t2")
                                nc.vector.tensor_tensor(
                                    t1[:], sw[:], sinb[:, ssl],
                                    op=mybir.AluOpType.mult)
                                nc.vector.tensor_tensor(
                                    t2[:], t[c][0:ROTc, ssl], cosb[:, ssl],
                                    op=mybir.AluOpType.mult)
                                nc.vector.tensor_add(
                                    t[c][0:ROTc, ssl], t1[:], t2[:])

                            # 4 Q/K passes (2 banks each) + 1 V pass (4 banks)
                            for pi, (we, dst, hf) in enumerate((
                                    (wq_e, QT, 0), (wq_e, QT, 1),
                                    (wk_e, KT, 0), (wk_e, KT, 1))):
                                bk = (pi % 2) * 2
                                t0 = pjps.tile([128, SC], fp32, tag=f"pj{bk}")
                                t1_ = pjps.tile([128, SC], fp32, tag=f"pj{bk + 1}")
                                for g in range(NG):
                                    wa = wqkp.tile([128, 8, HDLc // 2], bf16,
                                                   tag="wa")
                                    nc.sync.dma_start(wa[:], we[hf, g])
                                    for j in range(8):
                                        dc = g * 8 + j
                                        st_, sp_ = (dc == 0), (dc == NDC - 1)
                                        mm(t0[:], wa[:, j, 0:128], hst(dc),
                                           start=st_, stop=sp_)
                                        mm(t1_[:], wa[:, j, 128:256], hst(dc),
                                           start=st_, stop=sp_)
                                for j, ps in enumerate((t0, t1_)):
                                    c = hf * 2 + j
                                    if pi % 2 == 0:
                                        nc.scalar.copy(dst[c][:, ssl], ps[:])
                                    else:
                                        nc.vector.tensor_copy(dst[c][:, ssl], ps[:])
                                if hf == 0:
                                    rope(dst, 0)
                                else:
                                    rope(dst, 2)

                            # V pass: stationary = hst chunks, moving = wv
                            psv = [pjps.tile([128, HDLc], fp32, tag=f"pj{ss}", name=f"psv{ss}")
                                   for ss in range(4)]
                            for g in range(NG):
                                wvt = wvp.tile([128, 8, HDLc], bf16, tag="wv")
                                nc.sync.dma_start(wvt[:], wv_e[g])
                                for j in range(8):
                                    dc = g * 8 + j
                                    st_, sp_ = (dc == 0), (dc == NDC - 1)
                                    for ss in range(4):
                                        mm(psv[ss][:],
                                           hst(dc)[:, ss * 128:(ss + 1) * 128],
                                           wvt[:, j, :], start=st_, stop=sp_)
                            for ss in range(4):
                                kcv = st * 4 + ss
                                if ss % 2 == 0:
                                    nc.scalar.copy(V[kcv][:], psv[ss][:])
                                else:
                                    nc.vector.tensor_copy(V[kcv][:], psv[ss][:])

                    # ============ phase B: attention (transposed scores) =====
                    ATN = [atnp.tile([128, Sc], bf16, tag=f"ATN{c}", name=f"ATN{c}")
                           for c in range(NHC)]
                    recip = rcpp.tile([128, NSC, 8], fp32, tag="recip")
                    with (
                        tc.tile_pool(name="ptb", bufs=1) as ptp,
                        tc.tile_pool(name="rsb", bufs=1) as rsbp,
                        tc.tile_pool(name="scps", bufs=1, space="PSUM") as scps,
                        tc.tile_pool(name="atps", bufs=1, space="PSUM") as atps,
                        tc.tile_pool(name="rsps", bufs=1, space="PSUM") as rsps,
                    ):
                        psRS = rsps.tile([128, SC], fp32, tag="rs0")
                        rrb = rsbp.tile([1, 8, SC], fp32, tag="rrec")

                        def emit_scores(h, qm, kc):
                            c0 = h * (HDc // 128)
                            qsl = slice(qm * SC, (qm + 1) * SC)
                            kcl = slice(kc * 128, (kc + 1) * 128)
                            ss = scps.tile([128, SC], fp32, tag=f"ss{kc % 2}")
                            mm(ss[:], KT[c0][:, kcl], QT[c0][:, qsl],
                               start=True, stop=False)
                            mm(ss[:], KT[c0 + 1][:, kcl], QT[c0 + 1][:, qsl],
                               start=False, stop=True)
                            return ss

                        for h in range(HPCc):
                            c0 = h * (HDc // 128)
                            for qm in range(NSC):
                                nkc = (qm + 1) * 4
                                at = [atps.tile([128, SC], fp32, tag=f"at{hh}", name=f"at{hh}")
                                      for hh in range(HDc // 128)]
                                ss_cur = emit_scores(h, qm, 0)
                                for kc in range(nkc):
                                    if kc // 4 == qm:   # diagonal macro tile
                                        nc.vector.tensor_add(
                                            ss_cur[:], ss_cur[:],
                                            masks[:, kc % 4, :])
                                    pt = ptp.tile([128, SC], bf16,
                                                  tag=f"pt{kc % 3}")
                                    nc.scalar.activation(
                                        pt[:], ss_cur[:],
                                        mybir.ActivationFunctionType.Exp,
                                        bias=0.0, scale=1.0 / 16.0)
                                    if kc + 1 < nkc:
                                        ss_cur = emit_scores(h, qm, kc + 1)
                                    st_, sp_ = (kc == 0), (kc == nkc - 1)
                                    for hh in range(HDc // 128):
                                        mm(at[hh][:],
                                           V[kc][:, h * HDc + hh * 128:
                                                 h * HDc + (hh + 1) * 128],
                                           pt[:], start=st_, stop=sp_)
                                    mm(psRS[0:1, :],
                                       ones[:], pt[:], start=st_, stop=sp_)
                                nc.vector.reciprocal(
                                    rrb[0:1, h * NSC + qm, :], psRS[0:1, :])
                                qsl = slice(qm * SC, (qm + 1) * SC)
                                for hh in range(HDc // 128):
                                    if hh == 0:
                                        nc.scalar.copy(ATN[c0 + hh][:, qsl],
                                                       at[hh][:])
                                    else:
                                        nc.vector.tensor_copy(
                                            ATN[c0 + hh][:, qsl], at[hh][:])

                        # rowsum reciprocals -> q-partition layout
                        rsb8 = rsbp.tile([8, SC], fp32, tag="rsb8")
                        for i in range(8):
                            nc.sync.dma_start(rsb8[i:i + 1, :], rrb[0:1, i, :])
                        psT = rsps.tile([128, NSC, 8], fp32, tag="rst")
                        for qs in range(NSC):
                            nc.tensor.transpose(
                                psT[:, qs, :],
                                rsb8[:, qs * 128:(qs + 1) * 128],
                                ident8[:])
                        nc.vector.tensor_copy(recip[:], psT[:])

                    # ============ phase C: out projection + chunked RS =======
                    with (
                        tc.tile_pool(name="wo", bufs=2) as wop,
                        tc.tile_pool(name="ysb", bufs=2) as ysbp,
                        tc.tile_pool(name="yps", bufs=2, space="PSUM") as ypsp,
                    ):
                        dst = y_part if use_collective else y_e
                        for oc in range(NOC):
                            ocl = slice(oc * SC, (oc + 1) * SC)
                            woc = wop.tile([128, NHC, SC], bf16, tag="woc")
                            nc.sync.dma_start(woc[:], wo_e[oc])
                            for sg in range(Sc // 128):
                                sgl = slice(sg * 128, (sg + 1) * 128)
                                ysb0 = ysbp.tile([128, SC], bf16, tag="y0")
                                ysb = ysbp.tile([128, SC], bf16, tag="yf")
                                for h in range(HPCc):
                                    yps = ypsp.tile([128, SC], fp32, tag=f"yp{h}")
                                    for cc in range(HDc // 128):
                                        c = h * (HDc // 128) + cc
                                        mm(yps[:], ATN[c][:, sgl], woc[:, c, :],
                                           start=(cc == 0), stop=(cc == 1))
                                    rap = recip[:, sg % NSC,
                                                h * NSC + sg // NSC:
                                                h * NSC + sg // NSC + 1]
                                    if h == 0:
                                        nc.scalar.mul(ysb0[:], yps[:], rap)
                                    else:
                                        nc.vector.scalar_tensor_tensor(
                                            out=ysb[:], in0=yps[:],
                                            scalar=rap, in1=ysb0[:],
                                            op0=mybir.AluOpType.mult,
                                            op1=mybir.AluOpType.add)
                                nc.sync.dma_start(
                                    dst[oc, b * Sc + sg * 128:
                                        b * Sc + (sg + 1) * 128, :],
                                    ysb[:])
                            if use_collective and b == Bc - 1:
                                nc.gpsimd.collective_compute(
                                    "ReduceScatter",
                                    mybir.AluOpType.add,
                                    replica_groups=[list(range(n_cores))],
                                    ins=[y_part[oc]],
                                    outs=[rs_out[oc]],
                                )
                                nc.sync.dma_start(y_e[:, ocl], rs_out[oc])

    nc.compile()
    return nc


# ---------------------------------------------------------------- host prep

def _sinusoidal_np(num_pos, dim):
    inv_freq = 1.0 / (10000.0 ** (np.arange(0, dim, 2, dtype=np.float32) / dim))
    t = np.arange(num_pos, dtype=np.float32)[:, None] * inv_freq[None, :]
    return np.cos(t).astype(np.float32), np.sin(t).astype(np.float32)


def _host_arrays(hs, Wq, Wk, Wv, Wo, position_ids, cfg, n_cores):
    """Build the shared + per-core input arrays (pre-swizzled, bf16)."""
    import ml_dtypes
    bf = ml_dtypes.bfloat16
    Bc, Sc, Dc, HPCc, HDc, ROTc = (
        cfg["B"], cfg["S"], cfg["D"], cfg["HPC"], cfg["HD"], cfg["ROT"])
    HDLc = HPCc * HDc
    NSCc, NDCc, NGc, NOCc, NHCc = Sc // SC, Dc // 128, Dc // 1024, Dc // SC, HDLc // 128

    # hs_s[b, st, hf, p, j, f] = hs[b, st*SC+f, (hf*16+j)*128+p]
    hs_s = np.ascontiguousarray(
        hs.reshape(Bc, NSCc, SC, 2, NDCc // 2, 128)
        .transpose(0, 1, 3, 5, 4, 2)).astype(bf)

    cos_t, sin_t = _sinusoidal_np(max(MAX_POS, Sc), ROTc)   # [P, ROT//2]
    pos = np.asarray(position_ids).astype(np.int64)         # [B, S]
    cosg = cos_t[pos]                                       # [B, S, 32]
    sing = sin_t[pos]
    cosb = np.repeat(cosg.transpose(0, 2, 1), 2, axis=1)    # [B, 64, S]
    sinb_r = np.repeat(sing.transpose(0, 2, 1), 2, axis=1)
    sgn = np.ones((ROTc, 1), np.float32)
    sgn[0::2] = -1.0
    sinb = np.ascontiguousarray(sinb_r * sgn).astype(bf)
    cosb = np.ascontiguousarray(cosb).astype(bf)

    # transposed causal masks for diagonal 512 macro tile: masksT[k, kc, q]
    masksT = np.zeros((128, 4, SC), np.float32)
    kk = np.arange(128)[:, None]
    qq = np.arange(SC)[None, :]
    for m in range(4):
        masksT[:, m, :] = np.where(m * 128 + kk <= qq, 0.0, NEG)
    masksT = masksT.astype(bf)

    pswap = np.zeros((128, ROTc), np.float32)
    for f in range(ROTc // 2):
        pswap[2 * f + 1, 2 * f] = 1.0
        pswap[2 * f, 2 * f + 1] = 1.0
    ident8 = np.eye(8, dtype=np.float32)
    onesc = np.ones((128, 1), np.float32).astype(bf)

    shared = dict(hs_s=hs_s, cosb=cosb, sinb=sinb, masksT=masksT,
                  pswap=pswap.astype(bf), ident8=ident8, onesc=onesc)

    def _wswz_qk(w):   # [HDLc(rows of W slice), Dc] -> [2, NG, 128, 8, HDLc//2]
        # w here is the [HDLc, Dc] row-slice of the full weight; stationary
        # layout wq_s[hf, g, p, j, f] = w[hf*256+f, (g*8+j)*128+p]
        return np.ascontiguousarray(
            w.reshape(2, HDLc // 2, NGc, 8, 128)
            .transpose(0, 2, 4, 3, 1)).astype(bf)

    def _wswz_v(w):    # -> [NG, 128, 8, HDLc];  wv_s[g,p,j,f] = w[f,(g*8+j)*128+p]
        return np.ascontiguousarray(
            w.reshape(HDLc, NGc, 8, 128).transpose(1, 3, 2, 0)).astype(bf)

    def _wswz_o(w):    # w: [Dc, HDLc] slice of Wo -> [NOC, 128, NHC, SC]
        # wo_s[oc, p, c, f] = w[oc*SC+f, c*128+p]
        return np.ascontiguousarray(
            w.reshape(NOCc, SC, NHCc, 128).transpose(0, 3, 2, 1)).astype(bf)

    per_core = []
    for c in range(n_cores):
        csl = slice(c * HDLc, (c + 1) * HDLc)
        per_core.append(dict(
            wq_s=_wswz_qk(np.asarray(Wq)[csl, :]),
            wk_s=_wswz_qk(np.asarray(Wk)[csl, :]),
            wv_s=_wswz_v(np.asarray(Wv)[csl, :]),
            wo_s=_wswz_o(np.asarray(Wo)[:, csl]),
            **shared,
        ))
    return per_core


def _numpy_reference(hidden_states, Wq, Wk, Wv, Wo, layer_past_k, layer_past_v,
                     attention_mask, position_ids, new_key_loc, new_value_loc,
                     valid_key_indices, valid_value_indices, bucket_size):
    """Slow but general fallback (mirrors reference.py in numpy fp32)."""
    hs = np.asarray(hidden_states, np.float32)
    Bc, Sc, Dc = hs.shape
    q = (hs @ np.asarray(Wq).T).reshape(Bc, Sc, NH, HD)
    k = (hs @ np.asarray(Wk).T).reshape(Bc, Sc, NH, HD)
    v = (hs @ np.asarray(Wv).T).reshape(Bc, Sc, NH, HD)

    cos_t, sin_t = _sinusoidal_np(MAX_POS, ROT)
    pos = np.asarray(position_ids).astype(np.int64)
    c_ = cos_t[pos][:, :, None, :]      # [B,S,1,32]
    s_ = sin_t[pos][:, :, None, :]

    def rot(x):
        xr = x[..., :ROT].reshape(Bc, Sc, NH, ROT // 2, 2)
        x0, x1 = xr[..., 0], xr[..., 1]
        o0 = c_ * x0 - s_ * x1
        o1 = s_ * x0 + c_ * x1
        out = np.stack([o0, o1], axis=-1).reshape(Bc, Sc, NH, ROT)
        return np.concatenate([out, x[..., ROT:]], axis=-1)

    q, k = rot(q), rot(k)
    nk = np.asarray(layer_past_k, np.float32).copy()
    nv = np.asarray(layer_past_v, np.float32).copy()
    nk[np.asarray(new_key_loc)] = k.reshape(Bc * Sc, 1, NH, HD)
    nv[np.asarray(new_value_loc)] = v.reshape(Bc * Sc, 1, NH, HD)
    kg = nk[np.asarray(valid_key_indices)].reshape(
        Bc, bucket_size, NH, HD).transpose(0, 2, 1, 3)
    vg = nv[np.asarray(valid_value_indices)].reshape(
        Bc, bucket_size, NH, HD).transpose(0, 2, 1, 3)
    qh = q.transpose(0, 2, 1, 3)
    scores = np.einsum("bhqd,bhkd->bhqk", qh, kg)
    causal = np.tril(np.ones((MAX_POS, MAX_POS), bool))[
        bucket_size - Sc:bucket_size, :bucket_size]
    scores = np.where(causal, scores, np.float32(np.finfo(np.float32).min))
    scores = scores / np.float32(np.sqrt(HD)) + np.asarray(attention_mask, np.float32)
    scores = scores - scores.max(-1, keepdims=True)
    p = np.exp(scores)
    p = p / p.sum(-1, keepdims=True)
    attn = np.einsum("bhqk,bhkd->bhqd", p, vg)
    attn = attn.transpose(0, 2, 1, 3).reshape(Bc, Sc, Dc)
    return (attn @ np.asarray(Wo).T).astype(np.float32)


def _fast_path_ok(layer_past_k, layer_past_v, attention_mask, new_key_loc,
                  new_value_loc, valid_key_indices, valid_value_indices,
                  bucket_size, hs_shape):
    Bc, Sc, Dc = hs_shape
    if (Bc, Sc, Dc) != (B, S, D) or int(bucket_size) != S:
        return False
    ar = np.arange(Bc * Sc)
    for idx in (new_key_loc, new_value_loc, valid_key_indices, valid_value_indices):
        a = np.asarray(idx)
        if a.shape != (Bc * Sc,) or not np.array_equal(a, ar):
            return False
    if np.any(np.asarray(attention_mask) != 0):
        return False
    return True


_NC_CACHE = {}


def _get_nc(use_collective=True):
    key = ("v2", use_collective)
    if key not in _NC_CACHE:
        _NC_CACHE[key] = build_nc(_cfg_full(), use_collective=use_collective,
                                  n_cores=N_CORES)
    return _NC_CACHE[key]


def _assemble(outs, use_collective):
    if use_collective:
        y = np.concatenate([np.asarray(o, np.float32) for o in outs], axis=0)
    else:
        acc = np.zeros((D // SC, B * S, SC), np.float32)
        for o in outs:
            acc += np.asarray(o, np.float32)
        y = np.concatenate([acc[i] for i in range(D // SC)], axis=1)
    return y


def kernel(**inputs):
    hs = np.asarray(inputs["hidden_states"], np.float32)
    fast = _fast_path_ok(
        inputs["layer_past_k"], inputs["layer_past_v"], inputs["attention_mask"],
        inputs["new_key_loc"], inputs["new_value_loc"],
        inputs["valid_key_indices"], inputs["valid_value_indices"],
        inputs["bucket_size"], hs.shape)
    if not fast:
        return _numpy_reference(**inputs)

    from concourse.bass_utils import run_bass_kernel_spmd

    use_collective = os.environ.get("KERNEL_NO_COLLECTIVE", "") != "1"
    nc = _get_nc(use_collective)
    in_maps = _host_arrays(
        hs, np.asarray(inputs["Wq"], np.float32),
        np.asarray(inputs["Wk"], np.float32),
        np.asarray(inputs["Wv"], np.float32),
        np.asarray(inputs["Wo"], np.float32),
        inputs["position_ids"], _cfg_full(), N_CORES)
    res = run_bass_kernel_spmd(nc, in_maps, list(range(N_CORES)))
    outs = [res.results[c]["y"] for c in range(N_CORES)]
    y = _assemble(outs, use_collective)
    return y.reshape(B, S, D).astype(np.float32)


# revision 19
# speedup vs baseline: 1.5363x; 1.1682x over previous
"""GPT-J attention (B=2, S=2048, D=4096, 16 heads x 256, partial RoPE 64) on 8 trn2 cores.

Tensor-parallel over heads: each core owns 2 heads (Wq/Wk/Wv column slices,
Wo row slice). Per-oc-column bf16 ReduceScatter sums the partial out-projections
and leaves each core a 512-token shard of [B*S, D]; host concatenates shards.

v2 kernel strategy (all matmul inputs bf16, PSUM accumulation fp32):
  - Phase A per b: QKV projections accumulate over ALL 32 d-chunks directly in
    PSUM (no SBUF partial-sum round trips). Five passes per 512-token tile
    (Q01, Q23, K01, K23, V) using <= 6 PSUM banks with evacuation (alternating
    ACT/DVE) pipelined behind the next pass. Weights and hsT streamed in
    host-pre-swizzled partition-contiguous blocks. RoPE folded in per tile.
  - Phase B: scores computed TRANSPOSED (ssT[k,q] = KT_chunk^T @ QT) so the
    exp output is directly P^T, feeding PV with V as stationary - no PE
    transposes, no P copies. Row sums via a ones-column matmul into PSUM
    (partitions 0/32/64/96 of 2 banks), then reciprocal + one small PE
    transpose gives recip in q-partition layout. Softmax normalization is
    deferred to the phase-C evacuation. Software-pipelined: scores(kc+1)
    emitted before PV(kc).
  - Phase C per b, oc-outer: y partials accumulate per (sg,h) in PSUM;
    h=0 evacuated by ACT (activation Copy with per-partition recip scale),
    h=1 by DVE fused mult-add. bf16 y_part written per oc block; after b=1
    finishes an oc block, a ReduceScatter(add) for just that [4096,512] block
    fires, overlapping the remaining compute.
"""

import os
import sys

import numpy as np

sys.path.insert(0, "/opt/trn_rl_repo")

# ---------------------------------------------------------------- constants
B = 2
S = 2048
D = 4096
NH = 16
HD = 256
ROT = 64
MAX_POS = 2048
N_CORES = 8
HPC = NH // N_CORES          # heads per core = 2
HDL = HPC * HD               # local head width = 512

SC = 512                     # token tile / q macro tile / moving width
NEG = -1.0e30


def _cfg_full():
    return dict(B=B, S=S, D=D, HPC=HPC, HD=HD, ROT=ROT)


# ---------------------------------------------------------------- bass build

def build_nc(cfg, use_collective=True, n_cores=N_CORES):
    import concourse.tile as tile
    from concourse import bacc, mybir

    fp32 = mybir.dt.float32
    bf16 = mybir.dt.bfloat16

    Bc, Sc, Dc, HPCc, HDc, ROTc = (
        cfg["B"], cfg["S"], cfg["D"], cfg["HPC"], cfg["HD"], cfg["ROT"])
    HDLc = HPCc * HDc                    # local head width (512)
    NHC = HDLc // 128                    # local hd chunks (4)
    NSC = Sc // SC                       # 512-token tiles per b (4)
    NDC = Dc // 128                      # d chunks (32)
    NG = NDC // 8                        # streamed weight groups (4)
    NOC = Dc // SC                       # out-proj column chunks (8)
    NKC = Sc // 128                      # k chunks per b (16)
    SHARD = (Bc * Sc) // n_cores if use_collective else Bc * Sc

    nc = bacc.Bacc(num_devices=n_cores)

    # inputs (per-core, host-pre-swizzled for contiguous DMA)
    hs_e = nc.declare_dram_parameter("hs_s", [Bc, NSC, 4, 128, NDC // 4, SC],
                                     bf16, isOutput=False)
    wq_e = nc.declare_dram_parameter("wq_s", [2, NG, 128, 8, HDLc // 2],
                                     bf16, isOutput=False)
    wk_e = nc.declare_dram_parameter("wk_s", [2, NG, 128, 8, HDLc // 2],
                                     bf16, isOutput=False)
    wv_e = nc.declare_dram_parameter("wv_s", [NG, 128, 8, HDLc],
                                     bf16, isOutput=False)
    wo_e = nc.declare_dram_parameter("wo_s", [NDC, 128, NDC, 128],
                                     bf16, isOutput=False)
    cos_e = nc.declare_dram_parameter("cosb", [Bc, ROTc, Sc], bf16, isOutput=False)
    sin_e = nc.declare_dram_parameter("sinb", [Bc, ROTc, Sc], bf16, isOutput=False)
    msk_e = nc.declare_dram_parameter("masksT", [128, 4, SC], bf16, isOutput=False)
    psw_e = nc.declare_dram_parameter("pswap", [128, ROTc], bf16, isOutput=False)
    one_e = nc.declare_dram_parameter("onesc", [128, 1], bf16, isOutput=False)
    onr_e = nc.declare_dram_parameter("onesr", [1, 128], fp32, isOutput=False)

    TPC = Sc // n_cores                  # tokens per core per batch (256)
    y_e = nc.declare_dram_parameter("y", [Dc, Bc * TPC], bf16, isOutput=True)
    a2a_in = [nc.dram_tensor(f"a2a_in{b}", [n_cores, 128, NHC, TPC], bf16)
              for b in range(Bc)]
    a2a_out = [nc.dram_tensor(f"a2a_out{b}", [n_cores, 128, NHC, TPC], bf16)
               for b in range(Bc)]
    rcp_in = [nc.dram_tensor(f"rcp_in{b}", [n_cores, HPCc, TPC], fp32)
              for b in range(Bc)]
    rcp_out = [nc.dram_tensor(f"rcp_out{b}", [n_cores, HPCc, TPC], fp32)
               for b in range(Bc)]

    def mm(ps, lhsT, rhs, start, stop):
        nc.tensor.matmul(ps, lhsT, rhs, start=start, stop=stop)

    with tile.TileContext(nc) as tc:
        with tc.tile_pool(name="const", bufs=1) as constp:
            masks = constp.tile([128, 4, SC], bf16)
            nc.sync.dma_start(masks[:], msk_e[:])
            pswap = constp.tile([128, ROTc], bf16)
            nc.sync.dma_start(pswap[:], psw_e[:])
            ones = constp.tile([128, 1], bf16)
            nc.sync.dma_start(ones[:], one_e[:])
            onesr = constp.tile([1, 128], fp32)
            nc.sync.dma_start(onesr[:], onr_e[:])

            with (
                tc.tile_pool(name="qkv", bufs=1) as qkvp,      # QT/KT/V one b
                tc.tile_pool(name="atn", bufs=1) as atnp,      # ATN one b
                tc.tile_pool(name="rcp", bufs=1) as rcpp,      # recip per b
                tc.tile_pool(name="xn", bufs=1) as xnp,        # received x
                tc.tile_pool(name="bcs", bufs=2) as bcsb,
                tc.tile_pool(name="bcps", bufs=1, space="PSUM") as bcps,
            ):
                xn = [xnp.tile([128, NHC, Bc * TPC], bf16, tag=f"xn{s}",
                               name=f"xn{s}") for s in range(n_cores)]
                for b in range(Bc):
                    # ============ phase A: QKV projection (PSUM-resident) ====
                    QT = [qkvp.tile([128, Sc], bf16, tag=f"QT{c}", name=f"QT{c}") for c in range(NHC)]
                    KT = [qkvp.tile([128, Sc], bf16, tag=f"KT{c}", name=f"KT{c}") for c in range(NHC)]
                    V = [qkvp.tile([128, HDLc], bf16, tag=f"V{k}", name=f"V{k}") for k in range(NKC)]

                    with (
                        tc.tile_pool(name="trig", bufs=1) as trigp,
                        tc.tile_pool(name="hst", bufs=5) as hp,
                        tc.tile_pool(name="wqk", bufs=3) as wqkp,
                        tc.tile_pool(name="wvs", bufs=2) as wvp,
                        tc.tile_pool(name="pjps", bufs=1, space="PSUM") as pjps,
                        tc.tile_pool(name="rops", bufs=2, space="PSUM") as ropsp,
                        tc.tile_pool(name="ropb", bufs=2) as ropbp,
                    ):
                        cosb = trigp.tile([ROTc, Sc], bf16, tag="cos")
                        sinb = trigp.tile([ROTc, Sc], bf16, tag="sin")
                        nc.sync.dma_start(cosb[:], cos_e[b])
                        nc.sync.dma_start(sinb[:], sin_e[b])
                        HQD = NDC // 4
                        for st in range(NSC):
                            ssl = slice(st * SC, (st + 1) * SC)
                            hq = []
                            for q4 in range(4):
                                hq.append(hp.tile([128, HQD, SC], bf16,
                                                  tag="hst", name="hst"))
                                nc.sync.dma_start(hq[q4][:], hs_e[b, st, q4])

                            def hst(dc, _hq=hq):
                                return _hq[dc // HQD][:, dc % HQD, :]

                            def rope(t, c):
                                # rotate rows 0:ROT of t[c] at columns ssl
                                sw = ropsp.tile([ROTc, SC], fp32, tag="rp")
                                mm(sw[:], pswap[:, :], t[c][:, ssl],
                                   start=True, stop=True)
                                t1 = ropbp.tile([ROTc, SC], bf16, tag="t1")
                                t2 = ropbp.tile([ROTc, SC], bf16, tag="t2")
                                nc.vector.tensor_tensor(
                                    t1[:], sw[:], sinb[:, ssl],
                                    op=mybir.AluOpType.mult)
                                nc.vector.tensor_tensor(
                                    t2[:], t[c][0:ROTc, ssl], cosb[:, ssl],
                                    op=mybir.AluOpType.mult)
                                nc.vector.tensor_add(
                                    t[c][0:ROTc, ssl], t1[:], t2[:])

                            # 4 Q/K passes (2 banks each) + 1 V pass (4 banks)
                            for pi, (we, dst, hf) in enumerate((
                                    (wq_e, QT, 0), (wq_e, QT, 1),
                                    (wk_e, KT, 0), (wk_e, KT, 1))):
                                bk = (pi % 2) * 2
                                t0 = pjps.tile([128, SC], fp32, tag=f"pj{bk}")
                                t1_ = pjps.tile([128, SC], fp32, tag=f"pj{bk + 1}")
                                for g in range(NG):
                                    wa = wqkp.tile([128, 8, HDLc // 2], bf16,
                                                   tag="wa")
                                    nc.sync.dma_start(wa[:], we[hf, g])
                                    for j in range(8):
                                        dc = g * 8 + j
                                        st_, sp_ = (dc == 0), (dc == NDC - 1)
                                        mm(t0[:], wa[:, j, 0:128], hst(dc),
                                           start=st_, stop=sp_)
                                        mm(t1_[:], wa[:, j, 128:256], hst(dc),
                                           start=st_, stop=sp_)
                                for j, ps in enumerate((t0, t1_)):
                                    c = hf * 2 + j
                                    if pi % 2 == 0:
                                        nc.scalar.copy(dst[c][:, ssl], ps[:])
                                    else:
                                        nc.vector.tensor_copy(dst[c][:, ssl], ps[:])
                                if hf == 0:
                                    rope(dst, 0)
                                else:
                                    rope(dst, 2)

                            # V pass: stationary = hst chunks, moving = wv
                            psv = [pjps.tile([128, HDLc], fp32, tag=f"pj{ss}", name=f"psv{ss}")
                                   for ss in range(4)]
                            for g in range(NG):
                                wvt = wvp.tile([128, 8, HDLc], bf16, tag="wv")
                                nc.sync.dma_start(wvt[:], wv_e[g])
                                for j in range(8):
                                    dc = g * 8 + j
                                    st_, sp_ = (dc == 0), (dc == NDC - 1)
                                    for ss in range(4):
                                        mm(psv[ss][:],
                                           hst(dc)[:, ss * 128:(ss + 1) * 128],
                                           wvt[:, j, :], start=st_, stop=sp_)
                            for ss in range(4):
                                kcv = st * 4 + ss
                                if ss % 2 == 0:
                                    nc.scalar.copy(V[kcv][:], psv[ss][:])
                                else:
                                    nc.vector.tensor_copy(V[kcv][:], psv[ss][:])

                    # ============ phase B: attention (transposed scores) =====
                    ATN = [atnp.tile([128, Sc], bf16, tag=f"ATN{c}", name=f"ATN{c}")
                           for c in range(NHC)]
                    
                    with (
                        tc.tile_pool(name="ptb", bufs=1) as ptp,
                        tc.tile_pool(name="rsb", bufs=1) as rsbp,
                        tc.tile_pool(name="scps", bufs=1, space="PSUM") as scps,
                        tc.tile_pool(name="atps", bufs=1, space="PSUM") as atps,
                        tc.tile_pool(name="rsps", bufs=1, space="PSUM") as rsps,
                    ):
                        psRS = rsps.tile([128, SC], fp32, tag="rs0")
                        rrb = rsbp.tile([1, 8, SC], fp32, tag="rrec")

                        def emit_scores(h, qm, kc):
                            c0 = h * (HDc // 128)
                            qsl = slice(qm * SC, (qm + 1) * SC)
                            kcl = slice(kc * 128, (kc + 1) * 128)
                            ss = scps.tile([128, SC], fp32, tag=f"ss{kc % 2}")
                            mm(ss[:], KT[c0][:, kcl], QT[c0][:, qsl],
                               start=True, stop=False)
                            mm(ss[:], KT[c0 + 1][:, kcl], QT[c0 + 1][:, qsl],
                               start=False, stop=True)
                            return ss

                        for h in range(HPCc):
                            c0 = h * (HDc // 128)
                            for qm in range(NSC):
                                nkc = (qm + 1) * 4
                                at = [atps.tile([128, SC], fp32, tag=f"at{hh}", name=f"at{hh}")
                                      for hh in range(HDc // 128)]
                                ss_cur = emit_scores(h, qm, 0)
                                for kc in range(nkc):
                                    if kc // 4 == qm:   # diagonal macro tile
                                        nc.vector.tensor_add(
                                            ss_cur[:], ss_cur[:],
                                            masks[:, kc % 4, :])
                                    pt = ptp.tile([128, SC], bf16,
                                                  tag=f"pt{kc % 3}")
                                    nc.scalar.activation(
                                        pt[:], ss_cur[:],
                                        mybir.ActivationFunctionType.Exp,
                                        bias=0.0, scale=1.0 / 16.0)
                                    if kc + 1 < nkc:
                                        ss_cur = emit_scores(h, qm, kc + 1)
                                    st_, sp_ = (kc == 0), (kc == nkc - 1)
                                    for hh in range(HDc // 128):
                                        mm(at[hh][:],
                                           V[kc][:, h * HDc + hh * 128:
                                                 h * HDc + (hh + 1) * 128],
                                           pt[:], start=st_, stop=sp_)
                                    mm(psRS[0:1, :],
                                       ones[:], pt[:], start=st_, stop=sp_)
                                nc.vector.reciprocal(
                                    rrb[0:1, h * NSC + qm, :], psRS[0:1, :])
                                qsl = slice(qm * SC, (qm + 1) * SC)
                                for hh in range(HDc // 128):
                                    if hh == 0:
                                        nc.scalar.copy(ATN[c0 + hh][:, qsl],
                                                       at[hh][:])
                                    else:
                                        nc.vector.tensor_copy(
                                            ATN[c0 + hh][:, qsl], at[hh][:])

                        # ship ATN token-slices + recips to owning cores
                        for r in range(n_cores):
                            for c in range(NHC):
                                nc.sync.dma_start(
                                    a2a_in[b][r, :, c, :],
                                    ATN[c][:, r * TPC:(r + 1) * TPC])
                            for h in range(HPCc):
                                qm = (r * TPC) // SC
                                off = (r * TPC) % SC
                                nc.sync.dma_start(
                                    rcp_in[b][r, h:h + 1, :],
                                    rrb[0:1, h * NSC + qm, off:off + TPC])
                        if use_collective:
                            nc.gpsimd.collective_compute(
                                "AllToAll", mybir.AluOpType.bypass,
                                replica_groups=[list(range(n_cores))],
                                ins=[a2a_in[b][:]], outs=[a2a_out[b][:]])
                            nc.gpsimd.collective_compute(
                                "AllToAll", mybir.AluOpType.bypass,
                                replica_groups=[list(range(n_cores))],
                                ins=[rcp_in[b][:]], outs=[rcp_out[b][:]])
                        else:
                            nc.sync.dma_start(a2a_out[b][:], a2a_in[b][:])
                            nc.sync.dma_start(rcp_out[b][:], rcp_in[b][:])

                        # receive + normalize into xn columns for this b
                        rrt = rcpp.tile([1, n_cores, HPCc, TPC], fp32,
                                        tag="rrt", name="rrt")
                        nc.sync.dma_start(rrt[:], rcp_out[b][:])
                        bsl = slice(b * TPC, (b + 1) * TPC)
                        for s in range(n_cores):
                            nc.sync.dma_start(xn[s][:, :, bsl], a2a_out[b][s])
                        for s in range(n_cores):
                            for h in range(HPCc):
                                i2 = (s * HPCc + h) % 2
                                bc_ps = bcps.tile([128, TPC], fp32,
                                                  tag=f"bc{i2}", name=f"bc{i2}")
                                mm(bc_ps[:], onesr[:], rrt[0:1, s, h, :],
                                   start=True, stop=True)
                                bcs = bcsb.tile([128, TPC], bf16, tag=f"bcs{i2}",
                                                name=f"bcs{i2}")
                                nc.vector.tensor_copy(bcs[:], bc_ps[:])
                                for cc in range(HDc // 128):
                                    c = h * (HDc // 128) + cc
                                    nc.vector.tensor_tensor(
                                        xn[s][:, c, bsl], xn[s][:, c, bsl],
                                        bcs[:], op=mybir.AluOpType.mult)

                # ===== phase C': local out-projection over owned tokens ==
                with (
                    tc.tile_pool(name="wo2", bufs=4) as wo2p,
                    tc.tile_pool(name="ysbT", bufs=2) as ysbtp,
                    tc.tile_pool(name="ypsT", bufs=2, space="PSUM") as ypstp,
                ):
                    TT = Bc * TPC
                    for dsub in range(NDC):
                        wo2 = wo2p.tile([128, NDC, 128], bf16, tag="wo2")
                        nc.sync.dma_start(wo2[:], wo_e[dsub])
                        yT = ypstp.tile([128, TT], fp32, tag=f"yT{dsub % 2}",
                                        name=f"yT{dsub % 2}")
                        for cg in range(NDC):
                            s, c = cg // NHC, cg % NHC
                            mm(yT[:], wo2[:, cg, :], xn[s][:, c, :],
                               start=(cg == 0), stop=(cg == NDC - 1))
                        ysbT = ysbtp.tile([128, TT], bf16, tag="ysbT")
                        if dsub % 2 == 0:
                            nc.scalar.copy(ysbT[:], yT[:])
                        else:
                            nc.vector.tensor_copy(ysbT[:], yT[:])
                        nc.sync.dma_start(
                            y_e[dsub * 128:(dsub + 1) * 128, :], ysbT[:])

    nc.compile()
    return nc


# ---------------------------------------------------------------- host prep

def _sinusoidal_np(num_pos, dim):
    inv_freq = 1.0 / (10000.0 ** (np.arange(0, dim, 2, dtype=np.float32) / dim))
    t = np.arange(num_pos, dtype=np.float32)[:, None] * inv_freq[None, :]
    return np.cos(t).astype(np.float32), np.sin(t).astype(np.float32)


def _host_arrays(hs, Wq, Wk, Wv, Wo, position_ids, cfg, n_cores):
    """Build the shared + per-core input arrays (pre-swizzled, bf16)."""
    import ml_dtypes
    bf = ml_dtypes.bfloat16
    Bc, Sc, Dc, HPCc, HDc, ROTc = (
        cfg["B"], cfg["S"], cfg["D"], cfg["HPC"], cfg["HD"], cfg["ROT"])
    HDLc = HPCc * HDc
    NSCc, NDCc, NGc, NOCc, NHCc = Sc // SC, Dc // 128, Dc // 1024, Dc // SC, HDLc // 128

    # hs_s[b, st, hf, p, j, f] = hs[b, st*SC+f, (hf*16+j)*128+p]
    hs_s = np.ascontiguousarray(
        hs.reshape(Bc, NSCc, SC, 4, NDCc // 4, 128)
        .transpose(0, 1, 3, 5, 4, 2)).astype(bf)

    cos_t, sin_t = _sinusoidal_np(max(MAX_POS, Sc), ROTc)   # [P, ROT//2]
    pos = np.asarray(position_ids).astype(np.int64)         # [B, S]
    cosg = cos_t[pos]                                       # [B, S, 32]
    sing = sin_t[pos]
    cosb = np.repeat(cosg.transpose(0, 2, 1), 2, axis=1)    # [B, 64, S]
    sinb_r = np.repeat(sing.transpose(0, 2, 1), 2, axis=1)
    sgn = np.ones((ROTc, 1), np.float32)
    sgn[0::2] = -1.0
    sinb = np.ascontiguousarray(sinb_r * sgn).astype(bf)
    cosb = np.ascontiguousarray(cosb).astype(bf)

    # transposed causal masks for diagonal 512 macro tile: masksT[k, kc, q]
    masksT = np.zeros((128, 4, SC), np.float32)
    kk = np.arange(128)[:, None]
    qq = np.arange(SC)[None, :]
    for m in range(4):
        masksT[:, m, :] = np.where(m * 128 + kk <= qq, 0.0, NEG)
    masksT = masksT.astype(bf)

    pswap = np.zeros((128, ROTc), np.float32)
    for f in range(ROTc // 2):
        pswap[2 * f + 1, 2 * f] = 1.0
        pswap[2 * f, 2 * f + 1] = 1.0
    onesc = np.ones((128, 1), np.float32).astype(bf)
    onesr = np.ones((1, 128), np.float32)

    # wo2_s[dsub, p, cg, m] = Wo[dsub*128+m, cg*128+p] (full Wo, shared)
    wo2_s = np.ascontiguousarray(
        np.asarray(Wo).reshape(NDCc, 128, NDCc, 128)
        .transpose(0, 3, 2, 1)).astype(bf)

    shared = dict(hs_s=hs_s, cosb=cosb, sinb=sinb, masksT=masksT,
                  pswap=pswap.astype(bf), onesc=onesc, onesr=onesr,
                  wo_s=wo2_s)

    def _wswz_qk(w):   # [HDLc(rows of W slice), Dc] -> [2, NG, 128, 8, HDLc//2]
        # w here is the [HDLc, Dc] row-slice of the full weight; stationary
        # layout wq_s[hf, g, p, j, f] = w[hf*256+f, (g*8+j)*128+p]
        return np.ascontiguousarray(
            w.reshape(2, HDLc // 2, NGc, 8, 128)
            .transpose(0, 2, 4, 3, 1)).astype(bf)

    def _wswz_v(w):    # -> [NG, 128, 8, HDLc];  wv_s[g,p,j,f] = w[f,(g*8+j)*128+p]
        return np.ascontiguousarray(
            w.reshape(HDLc, NGc, 8, 128).transpose(1, 3, 2, 0)).astype(bf)

    per_core = []
    for c in range(n_cores):
        csl = slice(c * HDLc, (c + 1) * HDLc)
        per_core.append(dict(
            wq_s=_wswz_qk(np.asarray(Wq)[csl, :]),
            wk_s=_wswz_qk(np.asarray(Wk)[csl, :]),
            wv_s=_wswz_v(np.asarray(Wv)[csl, :]),
            **shared,
        ))
    return per_core


def _numpy_reference(hidden_states, Wq, Wk, Wv, Wo, layer_past_k, layer_past_v,
                     attention_mask, position_ids, new_key_loc, new_value_loc,
                     valid_key_indices, valid_value_indices, bucket_size):
    """Slow but general fallback (mirrors reference.py in numpy fp32)."""
    hs = np.asarray(hidden_states, np.float32)
    Bc, Sc, Dc = hs.shape
    q = (hs @ np.asarray(Wq).T).reshape(Bc, Sc, NH, HD)
    k = (hs @ np.asarray(Wk).T).reshape(Bc, Sc, NH, HD)
    v = (hs @ np.asarray(Wv).T).reshape(Bc, Sc, NH, HD)

    cos_t, sin_t = _sinusoidal_np(MAX_POS, ROT)
    pos = np.asarray(position_ids).astype(np.int64)
    c_ = cos_t[pos][:, :, None, :]      # [B,S,1,32]
    s_ = sin_t[pos][:, :, None, :]

    def rot(x):
        xr = x[..., :ROT].reshape(Bc, Sc, NH, ROT // 2, 2)
        x0, x1 = xr[..., 0], xr[..., 1]
        o0 = c_ * x0 - s_ * x1
        o1 = s_ * x0 + c_ * x1
        out = np.stack([o0, o1], axis=-1).reshape(Bc, Sc, NH, ROT)
        return np.concatenate([out, x[..., ROT:]], axis=-1)

    q, k = rot(q), rot(k)
    nk = np.asarray(layer_past_k, np.float32).copy()
    nv = np.asarray(layer_past_v, np.float32).copy()
    nk[np.asarray(new_key_loc)] = k.reshape(Bc * Sc, 1, NH, HD)
    nv[np.asarray(new_value_loc)] = v.reshape(Bc * Sc, 1, NH, HD)
    kg = nk[np.asarray(valid_key_indices)].reshape(
        Bc, bucket_size, NH, HD).transpose(0, 2, 1, 3)
    vg = nv[np.asarray(valid_value_indices)].reshape(
        Bc, bucket_size, NH, HD).transpose(0, 2, 1, 3)
    qh = q.transpose(0, 2, 1, 3)
    scores = np.einsum("bhqd,bhkd->bhqk", qh, kg)
    causal = np.tril(np.ones((MAX_POS, MAX_POS), bool))[
        bucket_size - Sc:bucket_size, :bucket_size]
    scores = np.where(causal, scores, np.float32(np.finfo(np.float32).min))
    scores = scores / np.float32(np.sqrt(HD)) + np.asarray(attention_mask, np.float32)
    scores = scores - scores.max(-1, keepdims=True)
    p = np.exp(scores)
    p = p / p.sum(-1, keepdims=True)
    attn = np.einsum("bhqk,bhkd->bhqd", p, vg)
    attn = attn.transpose(0, 2, 1, 3).reshape(Bc, Sc, Dc)
    return (attn @ np.asarray(Wo).T).astype(np.float32)


def _fast_path_ok(layer_past_k, layer_past_v, attention_mask, new_key_loc,
                  new_value_loc, valid_key_indices, valid_value_indices,
                  bucket_size, hs_shape):
    Bc, Sc, Dc = hs_shape
    if (Bc, Sc, Dc) != (B, S, D) or int(bucket_size) != S:
        return False
    ar = np.arange(Bc * Sc)
    for idx in (new_key_loc, new_value_loc, valid_key_indices, valid_value_indices):
        a = np.asarray(idx)
        if a.shape != (Bc * Sc,) or not np.array_equal(a, ar):
            return False
    if np.any(np.asarray(attention_mask) != 0):
        return False
    return True


_NC_CACHE = {}


def _get_nc(use_collective=True):
    key = ("v2", use_collective)
    if key not in _NC_CACHE:
        _NC_CACHE[key] = build_nc(_cfg_full(), use_collective=use_collective,
                                  n_cores=N_CORES)
    return _NC_CACHE[key]


def _assemble(outs, use_collective):
    # core r returns y^T [D, B*TPC]; its tokens are b*S + r*TPC + i
    TPC = S // len(outs)
    stk = np.stack([np.asarray(o, np.float32) for o in outs])  # [R, D, B*TPC]
    R = stk.shape[0]
    y = stk.reshape(R, D, B, TPC).transpose(2, 0, 3, 1).reshape(B * S, D)
    return y


def kernel(**inputs):
    hs = np.asarray(inputs["hidden_states"], np.float32)
    fast = _fast_path_ok(
        inputs["layer_past_k"], inputs["layer_past_v"], inputs["attention_mask"],
        inputs["new_key_loc"], inputs["new_value_loc"],
        inputs["valid_key_indices"], inputs["valid_value_indices"],
        inputs["bucket_size"], hs.shape)
    if not fast:
        return _numpy_reference(**inputs)

    from concourse.bass_utils import run_bass_kernel_spmd

    use_collective = os.environ.get("KERNEL_NO_COLLECTIVE", "") != "1"
    nc = _get_nc(use_collective)
    in_maps = _host_arrays(
        hs, np.asarray(inputs["Wq"], np.float32),
        np.asarray(inputs["Wk"], np.float32),
        np.asarray(inputs["Wv"], np.float32),
        np.asarray(inputs["Wo"], np.float32),
        inputs["position_ids"], _cfg_full(), N_CORES)
    res = run_bass_kernel_spmd(nc, in_maps, list(range(N_CORES)))
    outs = [res.results[c]["y"] for c in range(N_CORES)]
    y = _assemble(outs, use_collective)
    return y.reshape(B, S, D).astype(np.float32)


# revision 20
# speedup vs baseline: 1.6234x; 1.0567x over previous
"""GPT-J attention (B=2, S=2048, D=4096, 16 heads x 256, partial RoPE 64) on 8 trn2 cores.

Tensor-parallel over heads: each core owns 2 heads (Wq/Wk/Wv column slices,
Wo row slice). Per-oc-column bf16 ReduceScatter sums the partial out-projections
and leaves each core a 512-token shard of [B*S, D]; host concatenates shards.

v2 kernel strategy (all matmul inputs bf16, PSUM accumulation fp32):
  - Phase A per b: QKV projections accumulate over ALL 32 d-chunks directly in
    PSUM (no SBUF partial-sum round trips). Five passes per 512-token tile
    (Q01, Q23, K01, K23, V) using <= 6 PSUM banks with evacuation (alternating
    ACT/DVE) pipelined behind the next pass. Weights and hsT streamed in
    host-pre-swizzled partition-contiguous blocks. RoPE folded in per tile.
  - Phase B: scores computed TRANSPOSED (ssT[k,q] = KT_chunk^T @ QT) so the
    exp output is directly P^T, feeding PV with V as stationary - no PE
    transposes, no P copies. Row sums via a ones-column matmul into PSUM
    (partitions 0/32/64/96 of 2 banks), then reciprocal + one small PE
    transpose gives recip in q-partition layout. Softmax normalization is
    deferred to the phase-C evacuation. Software-pipelined: scores(kc+1)
    emitted before PV(kc).
  - Phase C per b, oc-outer: y partials accumulate per (sg,h) in PSUM;
    h=0 evacuated by ACT (activation Copy with per-partition recip scale),
    h=1 by DVE fused mult-add. bf16 y_part written per oc block; after b=1
    finishes an oc block, a ReduceScatter(add) for just that [4096,512] block
    fires, overlapping the remaining compute.
"""

import os
import sys

import numpy as np

sys.path.insert(0, "/opt/trn_rl_repo")

# ---------------------------------------------------------------- constants
B = 2
S = 2048
D = 4096
NH = 16
HD = 256
ROT = 64
MAX_POS = 2048
N_CORES = 8
HPC = NH // N_CORES          # heads per core = 2
HDL = HPC * HD               # local head width = 512

SC = 512                     # token tile / q macro tile / moving width
NEG = -1.0e30


def _cfg_full():
    return dict(B=B, S=S, D=D, HPC=HPC, HD=HD, ROT=ROT)


# ---------------------------------------------------------------- bass build

def build_nc(cfg, use_collective=True, n_cores=N_CORES):
    import concourse.tile as tile
    from concourse import bacc, mybir

    fp32 = mybir.dt.float32
    bf16 = mybir.dt.bfloat16

    Bc, Sc, Dc, HPCc, HDc, ROTc = (
        cfg["B"], cfg["S"], cfg["D"], cfg["HPC"], cfg["HD"], cfg["ROT"])
    HDLc = HPCc * HDc                    # local head width (512)
    NHC = HDLc // 128                    # local hd chunks (4)
    NSC = Sc // SC                       # 512-token tiles per b (4)
    NDC = Dc // 128                      # d chunks (32)
    NG = NDC // 8                        # streamed weight groups (4)
    NOC = Dc // SC                       # out-proj column chunks (8)
    NKC = Sc // 128                      # k chunks per b (16)
    SHARD = (Bc * Sc) // n_cores if use_collective else Bc * Sc

    nc = bacc.Bacc(num_devices=n_cores)

    # inputs (per-core, host-pre-swizzled for contiguous DMA)
    hs_e = nc.declare_dram_parameter("hs_s", [Bc, NSC, 4, 128, NDC // 4, SC],
                                     bf16, isOutput=False)
    wq_e = nc.declare_dram_parameter("wq_s", [2, NG, 128, 8, HDLc // 2],
                                     bf16, isOutput=False)
    wk_e = nc.declare_dram_parameter("wk_s", [2, NG, 128, 8, HDLc // 2],
                                     bf16, isOutput=False)
    wv_e = nc.declare_dram_parameter("wv_s", [NG, 128, 8, HDLc],
                                     bf16, isOutput=False)
    wo_e = nc.declare_dram_parameter("wo_s", [NDC, 128, NDC, 128],
                                     bf16, isOutput=False)
    cos_e = nc.declare_dram_parameter("cosb", [Bc, ROTc, Sc], bf16, isOutput=False)
    sin_e = nc.declare_dram_parameter("sinb", [Bc, ROTc, Sc], bf16, isOutput=False)
    msk_e = nc.declare_dram_parameter("masksT", [128, 4, SC], bf16, isOutput=False)
    psw_e = nc.declare_dram_parameter("pswap", [128, ROTc], bf16, isOutput=False)
    one_e = nc.declare_dram_parameter("onesc", [128, 1], bf16, isOutput=False)
    onr_e = nc.declare_dram_parameter("onesr", [1, 128], fp32, isOutput=False)

    TPC = Sc // n_cores                  # tokens per core per batch (256)
    y_e = nc.declare_dram_parameter("y", [Dc, Bc * TPC], bf16, isOutput=True)
    a2a_in = [nc.dram_tensor(f"a2a_in{b}", [n_cores, 128, NHC, TPC], bf16)
              for b in range(Bc)]
    a2a_out = [nc.dram_tensor(f"a2a_out{b}", [n_cores, 128, NHC, TPC], bf16)
               for b in range(Bc)]
    rcp_in = [nc.dram_tensor(f"rcp_in{b}", [n_cores, HPCc, TPC], fp32)
              for b in range(Bc)]
    rcp_out = [nc.dram_tensor(f"rcp_out{b}", [n_cores, HPCc, TPC], fp32)
               for b in range(Bc)]

    def mm(ps, lhsT, rhs, start, stop):
        nc.tensor.matmul(ps, lhsT, rhs, start=start, stop=stop)

    with tile.TileContext(nc) as tc:
        with tc.tile_pool(name="const", bufs=1) as constp:
            masks = constp.tile([128, 4, SC], bf16)
            nc.sync.dma_start(masks[:], msk_e[:])
            pswap = constp.tile([128, ROTc], bf16)
            nc.sync.dma_start(pswap[:], psw_e[:])
            ones = constp.tile([128, 1], bf16)
            nc.sync.dma_start(ones[:], one_e[:])
            onesr = constp.tile([1, 128], fp32)
            nc.sync.dma_start(onesr[:], onr_e[:])

            with (
                tc.tile_pool(name="qkv", bufs=1) as qkvp,      # QT/KT/V one b
                tc.tile_pool(name="atn", bufs=1) as atnp,      # ATN one b
                tc.tile_pool(name="rcp", bufs=1) as rcpp,      # recip per b
                tc.tile_pool(name="xn", bufs=1) as xnp,        # received x
                tc.tile_pool(name="bcs", bufs=2) as bcsb,
                tc.tile_pool(name="bcps", bufs=1, space="PSUM") as bcps,
            ):
                xn = [xnp.tile([128, NHC, Bc * TPC], bf16, tag=f"xn{s}",
                               name=f"xn{s}") for s in range(n_cores)]

                def emit_recv(b):
                    # receive + normalize xn columns for batch half b; emitted
                    # at a point where the a2a for b is already (nearly) done
                    rrt = rcpp.tile([1, n_cores, HPCc, TPC], fp32,
                                    tag="rrt", name="rrt")
                    nc.sync.dma_start(rrt[:], rcp_out[b][:])
                    bsl = slice(b * TPC, (b + 1) * TPC)
                    for s in range(n_cores):
                        nc.sync.dma_start(xn[s][:, :, bsl], a2a_out[b][s])
                    for s in range(n_cores):
                        for h in range(HPCc):
                            i2 = (s * HPCc + h) % 2
                            bc_ps = bcps.tile([128, TPC], fp32,
                                              tag=f"bc{i2}", name=f"bc{i2}")
                            mm(bc_ps[:], onesr[:], rrt[0:1, s, h, :],
                               start=True, stop=True)
                            bcs = bcsb.tile([128, TPC], bf16, tag=f"bcs{i2}",
                                            name=f"bcs{i2}")
                            nc.vector.tensor_copy(bcs[:], bc_ps[:])
                            for cc in range(HDc // 128):
                                c = h * (HDc // 128) + cc
                                nc.vector.tensor_tensor(
                                    xn[s][:, c, bsl], xn[s][:, c, bsl],
                                    bcs[:], op=mybir.AluOpType.mult)

                for b in range(Bc):
                    # ============ phase A: QKV projection (PSUM-resident) ====
                    QT = [qkvp.tile([128, Sc], bf16, tag=f"QT{c}", name=f"QT{c}") for c in range(NHC)]
                    KT = [qkvp.tile([128, Sc], bf16, tag=f"KT{c}", name=f"KT{c}") for c in range(NHC)]
                    V = [qkvp.tile([128, HDLc], bf16, tag=f"V{k}", name=f"V{k}") for k in range(NKC)]

                    with (
                        tc.tile_pool(name="trig", bufs=1) as trigp,
                        tc.tile_pool(name="hst", bufs=5) as hp,
                        tc.tile_pool(name="wqk", bufs=3) as wqkp,
                        tc.tile_pool(name="wvs", bufs=2) as wvp,
                        tc.tile_pool(name="pjps", bufs=1, space="PSUM") as pjps,
                        tc.tile_pool(name="rops", bufs=2, space="PSUM") as ropsp,
                        tc.tile_pool(name="ropb", bufs=2) as ropbp,
                    ):
                        cosb = trigp.tile([ROTc, Sc], bf16, tag="cos")
                        sinb = trigp.tile([ROTc, Sc], bf16, tag="sin")
                        nc.sync.dma_start(cosb[:], cos_e[b])
                        nc.sync.dma_start(sinb[:], sin_e[b])
                        HQD = NDC // 4
                        for st in range(NSC):
                            ssl = slice(st * SC, (st + 1) * SC)
                            hq = []
                            for q4 in range(4):
                                hq.append(hp.tile([128, HQD, SC], bf16,
                                                  tag="hst", name="hst"))
                                nc.sync.dma_start(hq[q4][:], hs_e[b, st, q4])

                            def hst(dc, _hq=hq):
                                return _hq[dc // HQD][:, dc % HQD, :]

                            def rope(t, c):
                                # rotate rows 0:ROT of t[c] at columns ssl
                                sw = ropsp.tile([ROTc, SC], fp32, tag="rp")
                                mm(sw[:], pswap[:, :], t[c][:, ssl],
                                   start=True, stop=True)
                                t1 = ropbp.tile([ROTc, SC], bf16, tag="t1")
                                t2 = ropbp.tile([ROTc, SC], bf16, tag="t2")
                                nc.vector.tensor_tensor(
                                    t1[:], sw[:], sinb[:, ssl],
                                    op=mybir.AluOpType.mult)
                                nc.vector.tensor_tensor(
                                    t2[:], t[c][0:ROTc, ssl], cosb[:, ssl],
                                    op=mybir.AluOpType.mult)
                                nc.vector.tensor_add(
                                    t[c][0:ROTc, ssl], t1[:], t2[:])

                            # 4 Q/K passes (2 banks each) + 1 V pass (4 banks)
                            for pi, (we, dst, hf) in enumerate((
                                    (wq_e, QT, 0), (wq_e, QT, 1),
                                    (wk_e, KT, 0), (wk_e, KT, 1))):
                                bk = (pi % 2) * 2
                                t0 = pjps.tile([128, SC], fp32, tag=f"pj{bk}")
                                t1_ = pjps.tile([128, SC], fp32, tag=f"pj{bk + 1}")
                                for g in range(NG):
                                    wa = wqkp.tile([128, 8, HDLc // 2], bf16,
                                                   tag="wa")
                                    nc.sync.dma_start(wa[:], we[hf, g])
                                    for j in range(8):
                                        dc = g * 8 + j
                                        st_, sp_ = (dc == 0), (dc == NDC - 1)
                                        mm(t0[:], wa[:, j, 0:128], hst(dc),
                                           start=st_, stop=sp_)
                                        mm(t1_[:], wa[:, j, 128:256], hst(dc),
                                           start=st_, stop=sp_)
                                for j, ps in enumerate((t0, t1_)):
                                    c = hf * 2 + j
                                    if pi % 2 == 0:
                                        nc.scalar.copy(dst[c][:, ssl], ps[:])
                                    else:
                                        nc.vector.tensor_copy(dst[c][:, ssl], ps[:])
                                if hf == 0:
                                    rope(dst, 0)
                                else:
                                    rope(dst, 2)

                            # V pass: stationary = hst chunks, moving = wv
                            psv = [pjps.tile([128, HDLc], fp32, tag=f"pj{ss}", name=f"psv{ss}")
                                   for ss in range(4)]
                            for g in range(NG):
                                wvt = wvp.tile([128, 8, HDLc], bf16, tag="wv")
                                nc.sync.dma_start(wvt[:], wv_e[g])
                                for j in range(8):
                                    dc = g * 8 + j
                                    st_, sp_ = (dc == 0), (dc == NDC - 1)
                                    for ss in range(4):
                                        mm(psv[ss][:],
                                           hst(dc)[:, ss * 128:(ss + 1) * 128],
                                           wvt[:, j, :], start=st_, stop=sp_)
                            for ss in range(4):
                                kcv = st * 4 + ss
                                if ss % 2 == 0:
                                    nc.scalar.copy(V[kcv][:], psv[ss][:])
                                else:
                                    nc.vector.tensor_copy(V[kcv][:], psv[ss][:])

                    # ============ phase B: attention (transposed scores) =====
                    if b == 1:
                        emit_recv(0)
                    ATN = [atnp.tile([128, Sc], bf16, tag=f"ATN{c}", name=f"ATN{c}")
                           for c in range(NHC)]
                    
                    with (
                        tc.tile_pool(name="ptb", bufs=1) as ptp,
                        tc.tile_pool(name="rsb", bufs=1) as rsbp,
                        tc.tile_pool(name="scps", bufs=1, space="PSUM") as scps,
                        tc.tile_pool(name="atps", bufs=1, space="PSUM") as atps,
                        tc.tile_pool(name="rsps", bufs=1, space="PSUM") as rsps,
                    ):
                        psRS = rsps.tile([128, SC], fp32, tag="rs0")
                        rrb = rsbp.tile([1, 8, SC], fp32, tag="rrec")

                        def emit_scores(h, qm, kc):
                            c0 = h * (HDc // 128)
                            qsl = slice(qm * SC, (qm + 1) * SC)
                            kcl = slice(kc * 128, (kc + 1) * 128)
                            ss = scps.tile([128, SC], fp32, tag=f"ss{kc % 2}")
                            mm(ss[:], KT[c0][:, kcl], QT[c0][:, qsl],
                               start=True, stop=False)
                            mm(ss[:], KT[c0 + 1][:, kcl], QT[c0 + 1][:, qsl],
                               start=False, stop=True)
                            return ss

                        for h in range(HPCc):
                            c0 = h * (HDc // 128)
                            for qm in range(NSC):
                                nkc = (qm + 1) * 4
                                at = [atps.tile([128, SC], fp32, tag=f"at{hh}", name=f"at{hh}")
                                      for hh in range(HDc // 128)]
                                ss_cur = emit_scores(h, qm, 0)
                                for kc in range(nkc):
                                    if kc // 4 == qm:   # diagonal macro tile
                                        nc.vector.tensor_add(
                                            ss_cur[:], ss_cur[:],
                                            masks[:, kc % 4, :])
                                    pt = ptp.tile([128, SC], bf16,
                                                  tag=f"pt{kc % 3}")
                                    nc.scalar.activation(
                                        pt[:], ss_cur[:],
                                        mybir.ActivationFunctionType.Exp,
                                        bias=0.0, scale=1.0 / 16.0)
                                    if kc + 1 < nkc:
                                        ss_cur = emit_scores(h, qm, kc + 1)
                                    st_, sp_ = (kc == 0), (kc == nkc - 1)
                                    for hh in range(HDc // 128):
                                        mm(at[hh][:],
                                           V[kc][:, h * HDc + hh * 128:
                                                 h * HDc + (hh + 1) * 128],
                                           pt[:], start=st_, stop=sp_)
                                    mm(psRS[0:1, :],
                                       ones[:], pt[:], start=st_, stop=sp_)
                                nc.vector.reciprocal(
                                    rrb[0:1, h * NSC + qm, :], psRS[0:1, :])
                                qsl = slice(qm * SC, (qm + 1) * SC)
                                for hh in range(HDc // 128):
                                    if hh == 0:
                                        nc.scalar.copy(ATN[c0 + hh][:, qsl],
                                                       at[hh][:])
                                    else:
                                        nc.vector.tensor_copy(
                                            ATN[c0 + hh][:, qsl], at[hh][:])

                        # ship ATN token-slices + recips to owning cores
                        for r in range(n_cores):
                            for c in range(NHC):
                                nc.scalar.dma_start(
                                    a2a_in[b][r, :, c, :],
                                    ATN[c][:, r * TPC:(r + 1) * TPC])
                            for h in range(HPCc):
                                qm = (r * TPC) // SC
                                off = (r * TPC) % SC
                                nc.scalar.dma_start(
                                    rcp_in[b][r, h:h + 1, :],
                                    rrb[0:1, h * NSC + qm, off:off + TPC])
                        if use_collective:
                            nc.gpsimd.collective_compute(
                                "AllToAll", mybir.AluOpType.bypass,
                                replica_groups=[list(range(n_cores))],
                                ins=[a2a_in[b][:]], outs=[a2a_out[b][:]])
                            nc.gpsimd.collective_compute(
                                "AllToAll", mybir.AluOpType.bypass,
                                replica_groups=[list(range(n_cores))],
                                ins=[rcp_in[b][:]], outs=[rcp_out[b][:]])
                        else:
                            nc.sync.dma_start(a2a_out[b][:], a2a_in[b][:])
                            nc.sync.dma_start(rcp_out[b][:], rcp_in[b][:])


                emit_recv(1)
                # ===== phase C': local out-projection over owned tokens ==
                with (
                    tc.tile_pool(name="wo2", bufs=4) as wo2p,
                    tc.tile_pool(name="ysbT", bufs=2) as ysbtp,
                    tc.tile_pool(name="ypsT", bufs=2, space="PSUM") as ypstp,
                ):
                    TT = Bc * TPC
                    for dsub in range(NDC):
                        wo2 = wo2p.tile([128, NDC, 128], bf16, tag="wo2")
                        nc.sync.dma_start(wo2[:], wo_e[dsub])
                        yT = ypstp.tile([128, TT], fp32, tag=f"yT{dsub % 2}",
                                        name=f"yT{dsub % 2}")
                        for cg in range(NDC):
                            s, c = cg // NHC, cg % NHC
                            mm(yT[:], wo2[:, cg, :], xn[s][:, c, :],
                               start=(cg == 0), stop=(cg == NDC - 1))
                        ysbT = ysbtp.tile([128, TT], bf16, tag="ysbT")
                        if dsub % 2 == 0:
                            nc.scalar.copy(ysbT[:], yT[:])
                        else:
                            nc.vector.tensor_copy(ysbT[:], yT[:])
                        nc.sync.dma_start(
                            y_e[dsub * 128:(dsub + 1) * 128, :], ysbT[:])

    nc.compile()
    return nc


# ---------------------------------------------------------------- host prep

def _sinusoidal_np(num_pos, dim):
    inv_freq = 1.0 / (10000.0 ** (np.arange(0, dim, 2, dtype=np.float32) / dim))
    t = np.arange(num_pos, dtype=np.float32)[:, None] * inv_freq[None, :]
    return np.cos(t).astype(np.float32), np.sin(t).astype(np.float32)


def _host_arrays(hs, Wq, Wk, Wv, Wo, position_ids, cfg, n_cores):
    """Build the shared + per-core input arrays (pre-swizzled, bf16)."""
    import ml_dtypes
    bf = ml_dtypes.bfloat16
    Bc, Sc, Dc, HPCc, HDc, ROTc = (
        cfg["B"], cfg["S"], cfg["D"], cfg["HPC"], cfg["HD"], cfg["ROT"])
    HDLc = HPCc * HDc
    NSCc, NDCc, NGc, NOCc, NHCc = Sc // SC, Dc // 128, Dc // 1024, Dc // SC, HDLc // 128

    # hs_s[b, st, hf, p, j, f] = hs[b, st*SC+f, (hf*16+j)*128+p]
    hs_s = np.ascontiguousarray(
        hs.reshape(Bc, NSCc, SC, 4, NDCc // 4, 128)
        .transpose(0, 1, 3, 5, 4, 2)).astype(bf)

    cos_t, sin_t = _sinusoidal_np(max(MAX_POS, Sc), ROTc)   # [P, ROT//2]
    pos = np.asarray(position_ids).astype(np.int64)         # [B, S]
    cosg = cos_t[pos]                                       # [B, S, 32]
    sing = sin_t[pos]
    cosb = np.repeat(cosg.transpose(0, 2, 1), 2, axis=1)    # [B, 64, S]
    sinb_r = np.repeat(sing.transpose(0, 2, 1), 2, axis=1)
    sgn = np.ones((ROTc, 1), np.float32)
    sgn[0::2] = -1.0
    sinb = np.ascontiguousarray(sinb_r * sgn).astype(bf)
    cosb = np.ascontiguousarray(cosb).astype(bf)

    # transposed causal masks for diagonal 512 macro tile: masksT[k, kc, q]
    masksT = np.zeros((128, 4, SC), np.float32)
    kk = np.arange(128)[:, None]
    qq = np.arange(SC)[None, :]
    for m in range(4):
        masksT[:, m, :] = np.where(m * 128 + kk <= qq, 0.0, NEG)
    masksT = masksT.astype(bf)

    pswap = np.zeros((128, ROTc), np.float32)
    for f in range(ROTc // 2):
        pswap[2 * f + 1, 2 * f] = 1.0
        pswap[2 * f, 2 * f + 1] = 1.0
    onesc = np.ones((128, 1), np.float32).astype(bf)
    onesr = np.ones((1, 128), np.float32)

    # wo2_s[dsub, p, cg, m] = Wo[dsub*128+m, cg*128+p] (full Wo, shared)
    wo2_s = np.ascontiguousarray(
        np.asarray(Wo).reshape(NDCc, 128, NDCc, 128)
        .transpose(0, 3, 2, 1)).astype(bf)

    shared = dict(hs_s=hs_s, cosb=cosb, sinb=sinb, masksT=masksT,
                  pswap=pswap.astype(bf), onesc=onesc, onesr=onesr,
                  wo_s=wo2_s)

    def _wswz_qk(w):   # [HDLc(rows of W slice), Dc] -> [2, NG, 128, 8, HDLc//2]
        # w here is the [HDLc, Dc] row-slice of the full weight; stationary
        # layout wq_s[hf, g, p, j, f] = w[hf*256+f, (g*8+j)*128+p]
        return np.ascontiguousarray(
            w.reshape(2, HDLc // 2, NGc, 8, 128)
            .transpose(0, 2, 4, 3, 1)).astype(bf)

    def _wswz_v(w):    # -> [NG, 128, 8, HDLc];  wv_s[g,p,j,f] = w[f,(g*8+j)*128+p]
        return np.ascontiguousarray(
            w.reshape(HDLc, NGc, 8, 128).transpose(1, 3, 2, 0)).astype(bf)

    per_core = []
    for c in range(n_cores):
        csl = slice(c * HDLc, (c + 1) * HDLc)
        per_core.append(dict(
            wq_s=_wswz_qk(np.asarray(Wq)[csl, :]),
            wk_s=_wswz_qk(np.asarray(Wk)[csl, :]),
            wv_s=_wswz_v(np.asarray(Wv)[csl, :]),
            **shared,
        ))
    return per_core


def _numpy_reference(hidden_states, Wq, Wk, Wv, Wo, layer_past_k, layer_past_v,
                     attention_mask, position_ids, new_key_loc, new_value_loc,
                     valid_key_indices, valid_value_indices, bucket_size):
    """Slow but general fallback (mirrors reference.py in numpy fp32)."""
    hs = np.asarray(hidden_states, np.float32)
    Bc, Sc, Dc = hs.shape
    q = (hs @ np.asarray(Wq).T).reshape(Bc, Sc, NH, HD)
    k = (hs @ np.asarray(Wk).T).reshape(Bc, Sc, NH, HD)
    v = (hs @ np.asarray(Wv).T).reshape(Bc, Sc, NH, HD)

    cos_t, sin_t = _sinusoidal_np(MAX_POS, ROT)
    pos = np.asarray(position_ids).astype(np.int64)
    c_ = cos_t[pos][:, :, None, :]      # [B,S,1,32]
    s_ = sin_t[pos][:, :, None, :]

    def rot(x):
        xr = x[..., :ROT].reshape(Bc, Sc, NH, ROT // 2, 2)
        x0, x1 = xr[..., 0], xr[..., 1]
        o0 = c_ * x0 - s_ * x1
        o1 = s_ * x0 + c_ * x1
        out = np.stack([o0, o1], axis=-1).reshape(Bc, Sc, NH, ROT)
        return np.concatenate([out, x[..., ROT:]], axis=-1)

    q, k = rot(q), rot(k)
    nk = np.asarray(layer_past_k, np.float32).copy()
    nv = np.asarray(layer_past_v, np.float32).copy()
    nk[np.asarray(new_key_loc)] = k.reshape(Bc * Sc, 1, NH, HD)
    nv[np.asarray(new_value_loc)] = v.reshape(Bc * Sc, 1, NH, HD)
    kg = nk[np.asarray(valid_key_indices)].reshape(
        Bc, bucket_size, NH, HD).transpose(0, 2, 1, 3)
    vg = nv[np.asarray(valid_value_indices)].reshape(
        Bc, bucket_size, NH, HD).transpose(0, 2, 1, 3)
    qh = q.transpose(0, 2, 1, 3)
    scores = np.einsum("bhqd,bhkd->bhqk", qh, kg)
    causal = np.tril(np.ones((MAX_POS, MAX_POS), bool))[
        bucket_size - Sc:bucket_size, :bucket_size]
    scores = np.where(causal, scores, np.float32(np.finfo(np.float32).min))
    scores = scores / np.float32(np.sqrt(HD)) + np.asarray(attention_mask, np.float32)
    scores = scores - scores.max(-1, keepdims=True)
    p = np.exp(scores)
    p = p / p.sum(-1, keepdims=True)
    attn = np.einsum("bhqk,bhkd->bhqd", p, vg)
    attn = attn.transpose(0, 2, 1, 3).reshape(Bc, Sc, Dc)
    return (attn @ np.asarray(Wo).T).astype(np.float32)


def _fast_path_ok(layer_past_k, layer_past_v, attention_mask, new_key_loc,
                  new_value_loc, valid_key_indices, valid_value_indices,
                  bucket_size, hs_shape):
    Bc, Sc, Dc = hs_shape
    if (Bc, Sc, Dc) != (B, S, D) or int(bucket_size) != S:
        return False
    ar = np.arange(Bc * Sc)
    for idx in (new_key_loc, new_value_loc, valid_key_indices, valid_value_indices):
        a = np.asarray(idx)
        if a.shape != (Bc * Sc,) or not np.array_equal(a, ar):
            return False
    if np.any(np.asarray(attention_mask) != 0):
        return False
    return True


_NC_CACHE = {}


def _get_nc(use_collective=True):
    key = ("v2", use_collective)
    if key not in _NC_CACHE:
        _NC_CACHE[key] = build_nc(_cfg_full(), use_collective=use_collective,
                                  n_cores=N_CORES)
    return _NC_CACHE[key]


def _assemble(outs, use_collective):
    # core r returns y^T [D, B*TPC]; its tokens are b*S + r*TPC + i
    TPC = S // len(outs)
    stk = np.stack([np.asarray(o, np.float32) for o in outs])  # [R, D, B*TPC]
    R = stk.shape[0]
    y = stk.reshape(R, D, B, TPC).transpose(2, 0, 3, 1).reshape(B * S, D)
    return y


def kernel(**inputs):
    hs = np.asarray(inputs["hidden_states"], np.float32)
    fast = _fast_path_ok(
        inputs["layer_past_k"], inputs["layer_past_v"], inputs["attention_mask"],
        inputs["new_key_loc"], inputs["new_value_loc"],
        inputs["valid_key_indices"], inputs["valid_value_indices"],
        inputs["bucket_size"], hs.shape)
    if not fast:
        return _numpy_reference(**inputs)

    from concourse.bass_utils import run_bass_kernel_spmd

    use_collective = os.environ.get("KERNEL_NO_COLLECTIVE", "") != "1"
    nc = _get_nc(use_collective)
    in_maps = _host_arrays(
        hs, np.asarray(inputs["Wq"], np.float32),
        np.asarray(inputs["Wk"], np.float32),
        np.asarray(inputs["Wv"], np.float32),
        np.asarray(inputs["Wo"], np.float32),
        inputs["position_ids"], _cfg_full(), N_CORES)
    res = run_bass_kernel_spmd(nc, in_maps, list(range(N_CORES)))
    outs = [res.results[c]["y"] for c in range(N_CORES)]
    y = _assemble(outs, use_collective)
    return y.reshape(B, S, D).astype(np.float32)
